# revision 15
# baseline (speedup 1.0000x reference)
"""Trainium2 Bass kernel for nn_MultiHeadAttention_61357902791348.

Sharding: 12 heads on 8 cores. Core pair (2p, 2p+1) owns heads {3p, 3p+1}
fully and splits head 3p+2's query rows (even core: rows [0,1600), odd:
[1600,3200)) -- balanced head/sequence-hybrid tensor parallelism with no
device collectives. Each core emits partial out-projection results; the
host sums the 8 partials and adds bo.

v2 rewrite vs baseline:
  * all PE operands bf16 (moving-stream bytes halved; fp32 PSUM accum)
  * no [1,W] single-lane vector/scalar ops: RMS-norm and softmax-sum
    rows are transposed onto partitions (K=1 ones-matmul trick) before
    rsqrt/reciprocal
  * softmax normalization deferred to the out-projection PSUM->SBUF copy
    as a per-partition activation scale (tokens on partitions there)
  * attention software-pipelined: scores(jt+1) issued before sm/ao(jt)
    so the exp latency on ACT hides under PE work
  * rope(q chunk) -> attention(chunk) -> out-proj(chunk) interleaved in
    one loop so DVE rope work hides under attention PE work
"""

import numpy as np

B, N, D = 1, 3200, 1536
NH, HD = 12, 128
F, Hg, Wg = 8, 20, 20
EPS = 1e-6
NS = 1600          # shared-head query rows per core
NCH = D // 128     # 12 D-chunks
PW = 400           # projection moving tile width
RW = 512           # rope / attention chunk width
NQ = N + NS        # 4800 q tokens per core (own + shared)
NK = 2 * N         # 6400 k tokens per core (own + shared heads)
NCHUNK = 38        # ceil(NQ / 128) 128-token chunks (last is 64 wide)

_CACHE = {}


def subtiles(total, width):
    return [(o, min(width, total - o)) for o in range(0, total, width)]


def _build():
    import concourse.bacc as bacc
    import concourse.mybir as mybir
    import concourse.tile as tile

    F32 = mybir.dt.float32
    BF16 = mybir.dt.bfloat16
    AF = mybir.ActivationFunctionType

    nc = bacc.Bacc("TRN2", target_bir_lowering=False, debug=False)

    xT = nc.dram_tensor("xT", [D, N], BF16, kind="ExternalInput")
    w6 = nc.dram_tensor("w6", [6, D, HD], BF16, kind="ExternalInput")
    bias6 = nc.dram_tensor("bias6", [HD, 6], F32, kind="ExternalInput")
    cq = nc.dram_tensor("cq", [HD, N], BF16, kind="ExternalInput")
    sq = nc.dram_tensor("sq", [HD, N], BF16, kind="ExternalInput")
    ck = nc.dram_tensor("ck", [HD, N], BF16, kind="ExternalInput")
    sk = nc.dram_tensor("sk", [HD, N], BF16, kind="ExternalInput")
    rotm = nc.dram_tensor("rotm", [HD, HD], BF16, kind="ExternalInput")
    eye = nc.dram_tensor("eye", [HD, HD], BF16, kind="ExternalInput")
    ones2d = nc.dram_tensor("ones2d", [HD, HD], BF16, kind="ExternalInput")
    ones2df = nc.dram_tensor("ones2df", [HD, HD], F32, kind="ExternalInput")
    eyef = nc.dram_tensor("eyef", [HD, HD], F32, kind="ExternalInput")
    wo2 = nc.dram_tensor("wo2", [2, HD, D], BF16, kind="ExternalInput")
    y_own = nc.dram_tensor("y_own", [N, D], F32, kind="ExternalOutput")
    y_sh = nc.dram_tensor("y_sh", [NS, D], F32, kind="ExternalOutput")

    with tile.TileContext(nc) as tc:
        import contextlib

        stack = contextlib.ExitStack()
        with stack:
            persist = stack.enter_context(tc.tile_pool(name="persist", bufs=1))
            qT = persist.tile([128, NQ], BF16, tag="qT")
            kT = persist.tile([128, NK], BF16, tag="kT")
            vtok = persist.tile([128, 2, 25, HD], BF16, tag="vtok")
            aoT = persist.tile([128, NQ], BF16, tag="aoT")
            bias_sb = persist.tile([HD, 6], F32, tag="bias")
            nc.sync.dma_start(bias_sb[:], bias6[:])
            ones_sb = persist.tile([HD, HD], BF16, tag="ones")
            nc.sync.dma_start(ones_sb[:], ones2d[:])
            rot_sb = persist.tile([HD, HD], BF16, tag="rot")
            nc.sync.dma_start(rot_sb[:], rotm[:])
            eye_sb = persist.tile([HD, HD], BF16, tag="eye")
            nc.sync.dma_start(eye_sb[:], eye[:])
            ones_f = persist.tile([HD, HD], F32, tag="onesf")
            nc.sync.dma_start(ones_f[:], ones2df[:])
            eye_f = persist.tile([HD, HD], F32, tag="eyef")
            nc.sync.dma_start(eye_f[:], eyef[:])
            wo_sb = [persist.tile([HD, D], BF16, tag=f"wo{u}", name=f"wo_sb{u}")
                     for u in range(2)]
            for u in range(2):
                nc.sync.dma_start(wo_sb[u][:], wo2[u, :, :])
            # rope tables resident in SBUF (no per-tile DMAs in the hot loop)
            tabs = {}
            for nm, dr in (("cq", cq), ("sq", sq), ("ck", ck), ("sk", sk)):
                t = persist.tile([128, N], BF16, tag=f"tab_{nm}",
                                 name=f"tab_{nm}")
                nc.sync.dma_start(t[:], dr[:])
                tabs[nm] = t
            rsk_sb = persist.tile([128, 50], F32, tag="rsk")
            inv_sb = persist.tile([128, NCHUNK], F32, tag="inv")
            rows_all = persist.tile([1, NCHUNK * 128], F32, tag="rows_all")
            bias_q = persist.tile([128, 1], F32, tag="bias_q")
            nc.vector.memset(bias_q[:], HD * EPS)
            bias_k = persist.tile([128, 1], F32, tag="bias_k")
            nc.vector.memset(bias_k[:], EPS)

            row_ps = stack.enter_context(
                tc.tile_pool(name="row_ps", bufs=2, space="PSUM"))   # [1,512]
            smt_ps = stack.enter_context(
                tc.tile_pool(name="smt_ps", bufs=1, space="PSUM"))   # [128,8]
            rtmp = stack.enter_context(tc.tile_pool(name="rtmp", bufs=3))
            pools = {}

            def rms_rs(kind, src_row_psum, w, dst_f32, dst_col0, uid):
                """Transpose ssq row [1,w] onto partitions, rsqrt there."""
                ssq_row = rtmp.tile([1, RW], F32, tag="ssqr", name=f"sr{uid}")
                nc.scalar.activation(ssq_row[:, :w], src_row_psum, AF.Copy)
                smt = smt_ps.tile([128, 8], F32, tag="smt", name=f"smt{uid}")
                ncc = (w + 127) // 128
                for i in range(ncc):
                    lo = i * 128
                    ccw = min(128, w - lo)
                    nc.tensor.matmul(
                        smt[0:ccw, 2 * i:2 * i + 2],
                        ssq_row[0:1, lo:lo + ccw],
                        ones_f[0:1, 0:2],
                        start=True, stop=True,
                    )
                sq_t = rtmp.tile([128, 8], F32, tag="sqt", name=f"sq{uid}")
                if kind == "q":
                    nc.scalar.activation(
                        sq_t[:, :ncc], smt[:, 0:2 * ncc:2], AF.Sqrt,
                        bias=bias_q[:], scale=1.0,
                    )
                else:
                    nc.scalar.activation(
                        sq_t[:, :ncc], smt[:, 0:2 * ncc:2], AF.Sqrt,
                        bias=bias_k[:], scale=1.0 / HD,
                    )
                nc.vector.reciprocal(
                    dst_f32[:, dst_col0:dst_col0 + ncc], sq_t[:, :ncc])
                return ncc

            def rope_stats(kind, o, w, uid):
                """Sum-of-squares -> rs for one tile; q also flattens rs to
                a row in rows_all (for the later broadcast matmul)."""
                big = qT if kind == "q" else kT
                src = big[:, o:o + w]
                q2 = rtmp.tile([128, RW], BF16, tag="q2", name=f"q2{uid}")
                nc.vector.tensor_mul(q2[:, :w], src, src)
                ssq = row_ps.tile([1, RW], F32, tag="row", name=f"ssq{uid}")
                nc.tensor.matmul(
                    ssq[:, :w], ones_sb[:, 0:1], q2[:, :w],
                    start=True, stop=True,
                )
                if kind == "k":
                    rms_rs("k", ssq[:, :w], w, rsk_sb, o // 128, uid)
                    return
                rsq = rtmp.tile([128, 8], F32, tag="rsq", name=f"rsq{uid}")
                ncc = rms_rs("q", ssq[:, :w], w, rsq, 0, uid)
                trp = pools["trp"].tile([128, 128], F32, tag="trp",
                                        name=f"trp{uid}")
                nc.tensor.transpose(trp[0:ncc, :], rsq[:, 0:ncc], eye_f[:])
                rows8 = rtmp.tile([8, 128], F32, tag="rows8", name=f"r8{uid}")
                nc.vector.tensor_copy(rows8[0:ncc, :], trp[0:ncc, :])
                nc.sync.dma_start(
                    rows_all[0:1, o:o + ncc * 128], rows8[0:ncc, :])

            def rope_apply(kind, o, tok, w, uid, ps512):
                big = qT if kind == "q" else kT
                ctab = tabs["cq"] if kind == "q" else tabs["ck"]
                stab = tabs["sq"] if kind == "q" else tabs["sk"]
                src = big[:, o:o + w]
                rot = ps512.tile([128, RW], F32, tag="ps", name=f"rt{uid}")
                nc.tensor.matmul(
                    rot[:, :w], rot_sb[:], src, start=True, stop=True)
                if kind == "q":
                    bcp = ps512.tile([128, RW], F32, tag="ps", name=f"bc{uid}")
                    ncc = (w + 127) // 128
                    for i in range(ncc):
                        lo = i * 128
                        ccw = min(128, w - lo)
                        nc.tensor.matmul(
                            bcp[:, lo:lo + ccw], ones_f[0:1, :],
                            rows_all[0:1, o + lo:o + lo + ccw],
                            start=True, stop=True,
                        )
                m1 = rtmp.tile([128, RW], BF16, tag="m1", name=f"m1{uid}")
                nc.vector.tensor_mul(m1[:, :w], src, ctab[:, tok:tok + w])
                m2 = rtmp.tile([128, RW], BF16, tag="m2", name=f"m2{uid}")
                nc.vector.tensor_mul(m2[:, :w], rot[:, :w], stab[:, tok:tok + w])
                if kind == "k":
                    nc.vector.tensor_add(src, m1[:, :w], m2[:, :w])
                else:
                    qr = rtmp.tile([128, RW], BF16, tag="qr", name=f"qr{uid}")
                    nc.vector.tensor_add(qr[:, :w], m1[:, :w], m2[:, :w])
                    nc.vector.tensor_mul(src, qr[:, :w], bcp[:, :w])

            # ---------------- projection phase ----------------
            with tc.tile_pool(name="vt", bufs=1) as vt_pool:
                vT = vt_pool.tile([128, NK], BF16, tag="vT")
                with tc.tile_pool(name="xt", bufs=1) as xt_pool, \
                     tc.tile_pool(name="wld", bufs=4) as w_pool, \
                     tc.tile_pool(name="pp", bufs=4, space="PSUM") as pp:
                    for half in range(2):
                        h0 = half * 1600
                        xts = []
                        for c in range(NCH):
                            xt = xt_pool.tile([128, 1600], BF16, tag=f"xt{c}")
                            nc.sync.dma_start(
                                xt[:], xT[c * 128:(c + 1) * 128, h0:h0 + 1600])
                            xts.append(xt)
                        # blocks: 0 q_own, 1 q_sh, 2 k_own, 3 k_sh, 4 v_own, 5 v_sh
                        for b in range(6):
                            if b == 1 and half == 1:
                                continue
                            if b == 0:
                                dst, d0 = qT, h0
                            elif b == 1:
                                dst, d0 = qT, N + h0
                            elif b in (2, 3):
                                dst, d0 = kT, (b - 2) * N + h0
                            else:
                                dst, d0 = vT, (b - 4) * N + h0
                            wtiles = []
                            for c in range(NCH):
                                wt = w_pool.tile([128, HD], BF16, tag="w")
                                nc.sync.dma_start(
                                    wt[:], w6[b, c * 128:(c + 1) * 128, :])
                                wtiles.append(wt)
                            for (o, w) in subtiles(1600, PW):
                                ps = pp.tile([128, PW], F32, tag="pp")
                                for c in range(NCH):
                                    nc.tensor.matmul(
                                        ps[:, :w], wtiles[c][:],
                                        xts[c][:, o:o + w],
                                        start=(c == 0), stop=(c == NCH - 1),
                                    )
                                nc.vector.tensor_scalar_add(
                                    dst[:, d0 + o:d0 + o + w], ps[:, :w],
                                    bias_sb[:, b:b + 1],
                                )

                k_tiles = []
                for seg in range(2):
                    for (ol, w) in subtiles(N, RW):
                        k_tiles.append((seg * N + ol, ol, w))

                with tc.tile_pool(name="ps512", bufs=3, space="PSUM") as ps512:
                    # V transposes (5 per psum bank) interleaved with rope(k)
                    with tc.tile_pool(name="vtp", bufs=2, space="PSUM") as vtp:
                        for i in range(len(k_tiles)):
                            if i < 10:
                                h, g = divmod(i, 5)
                                tpg = vtp.tile([128, 5, HD], BF16, tag="tp",
                                               name=f"tp{i}")
                                for k5 in range(5):
                                    jt = g * 5 + k5
                                    nc.tensor.transpose(
                                        tpg[:, k5, :],
                                        vT[:, h * N + jt * 128:
                                           h * N + (jt + 1) * 128],
                                        eye_sb[:],
                                    )
                                nc.scalar.activation(
                                    vtok[:, h, g * 5:(g + 1) * 5, :], tpg[:],
                                    AF.Copy)
                            (o, tok, w) = k_tiles[i]
                            rope_stats("k", o, w, f"k{i}")
                            rope_apply("k", o, tok, w, f"k{i}", ps512)

                    chunks = []
                    for (ol, w) in subtiles(N, RW):
                        chunks.append((0, ol, ol, w))     # unit, qcol, tok, w
                    for (ol, w) in subtiles(NS, RW):
                        chunks.append((1, N + ol, ol, w))

                    with tc.tile_pool(name="aops", bufs=1, space="PSUM") as aops, \
                         tc.tile_pool(name="trp_ps", bufs=1, space="PSUM") as trp_ps, \
                         tc.tile_pool(name="expp", bufs=3) as expp, \
                         tc.tile_pool(name="accp", bufs=2) as accp, \
                         tc.tile_pool(name="yout", bufs=3) as yout:
                        pools["trp"] = trp_ps

                        # q-side RMS stats (Sqrt table) all before first Exp
                        for ci, (unit, gco, tok, cw) in enumerate(chunks):
                            rope_stats("q", gco, cw, f"q{ci}")

                        for ci, (unit, gco, tok, cw) in enumerate(chunks):
                            rope_apply("q", gco, tok, cw, f"q{ci}", ps512)

                            head = unit
                            ao = aops.tile([128, RW], F32, tag="ao",
                                           name=f"ao{ci}")
                            acc = accp.tile([128, RW], F32, tag="acc",
                                            name=f"acc{ci}")
                            prev = None
                            for jt in range(25):
                                gjt = head * 25 + jt
                                sc = ps512.tile([128, RW], F32, tag="ps",
                                                name=f"sc{ci}_{jt}")
                                nc.tensor.matmul(
                                    sc[:, :cw],
                                    kT[:, gjt * 128:(gjt + 1) * 128],
                                    qT[:, gco:gco + cw],
                                    start=True, stop=True,
                                )
                                ex = expp.tile([128, RW], BF16, tag="ex",
                                               name=f"ex{ci}_{jt}")
                                nc.scalar.activation(
                                    ex[:, :cw], sc[:, :cw], AF.Exp,
                                    scale=rsk_sb[:, gjt:gjt + 1],
                                )
                                # softmax sums accumulate on the idle Pool
                                # engine (f32), freeing the PE of sm matmuls
                                if jt == 0:
                                    nc.gpsimd.tensor_copy(
                                        acc[:, :cw], ex[:, :cw])
                                else:
                                    nc.gpsimd.tensor_add(
                                        acc[:, :cw], acc[:, :cw], ex[:, :cw])
                                if prev is not None:
                                    pex, pjt = prev
                                    nc.tensor.matmul(
                                        ao[:, :cw], vtok[:, head, pjt, :],
                                        pex[:, :cw],
                                        start=(pjt == 0), stop=False,
                                    )
                                prev = (ex, jt)
                            pex, pjt = prev
                            nc.tensor.matmul(
                                ao[:, :cw], vtok[:, head, pjt, :], pex[:, :cw],
                                start=False, stop=True,
                            )
                            # chunk tail: cross-partition sum of acc, invert
                            g0 = gco // 128
                            sm = row_ps.tile([1, RW], F32, tag="row",
                                             name=f"sm{ci}")
                            nc.tensor.matmul(
                                sm[:, :cw], ones_f[:, 0:1], acc[:, :cw],
                                start=True, stop=True,
                            )
                            smrow = rtmp.tile([1, RW], F32, tag="ssqr",
                                              name=f"smr{ci}")
                            nc.vector.tensor_copy(smrow[:, :cw], sm[:, :cw])
                            smt = smt_ps.tile([128, 8], F32, tag="smt",
                                              name=f"smT{ci}")
                            ncc = (cw + 127) // 128
                            for i in range(ncc):
                                lo = i * 128
                                ccw = min(128, cw - lo)
                                nc.tensor.matmul(
                                    smt[0:ccw, 2 * i:2 * i + 2],
                                    smrow[0:1, lo:lo + ccw],
                                    ones_f[0:1, 0:2],
                                    start=True, stop=True,
                                )
                            nc.vector.reciprocal(
                                inv_sb[:, g0:g0 + ncc], smt[:, 0:2 * ncc:2])
                            nc.vector.tensor_copy(
                                aoT[:, gco:gco + cw], ao[:, :cw])

                            # out-projection for this chunk
                            ydst = y_own if unit == 0 else y_sh
                            for (it, iw) in subtiles(cw, 128):
                                git = gco + it
                                gidx = git // 128
                                yt = yout.tile([128, D], F32, tag="yt",
                                               name=f"yt{ci}_{it}")
                                for ct3 in range(3):
                                    op = ps512.tile([128, RW], F32, tag="ps",
                                                    name=f"op{ci}_{it}_{ct3}")
                                    nc.tensor.matmul(
                                        op[0:iw, :], aoT[:, git:git + iw],
                                        wo_sb[unit][:, ct3 * 512:(ct3 + 1) * 512],
                                        start=True, stop=True,
                                    )
                                    if ct3 % 2 == 0:
                                        nc.vector.tensor_scalar_mul(
                                            yt[0:iw, ct3 * 512:(ct3 + 1) * 512],
                                            op[0:iw, :],
                                            inv_sb[0:iw, gidx:gidx + 1],
                                        )
                                    else:
                                        # Copy lives in every ACT table: no
                                        # act-table swap against Exp
                                        nc.scalar.activation(
                                            yt[0:iw, ct3 * 512:(ct3 + 1) * 512],
                                            op[0:iw, :], AF.Copy,
                                            scale=inv_sb[0:iw, gidx:gidx + 1],
                                        )
                                nc.sync.dma_start(
                                    ydst[tok + it:tok + it + iw, :], yt[0:iw, :])

    nc.compile()
    return nc


def _get_nc():
    if "nc" not in _CACHE:
        _CACHE["nc"] = _build()
    return _CACHE["nc"]


def _host_prep(inputs):
    import ml_dtypes

    bf16 = ml_dtypes.bfloat16
    x = np.asarray(inputs["x"], np.float32)[0]          # [N, D]
    Wq = np.asarray(inputs["Wq"], np.float32)
    Wk = np.asarray(inputs["Wk"], np.float32)
    Wv = np.asarray(inputs["Wv"], np.float32)
    Wo = np.asarray(inputs["Wo"], np.float32)
    bq = np.asarray(inputs["bq"], np.float32)
    bk = np.asarray(inputs["bk"], np.float32)
    bv = np.asarray(inputs["bv"], np.float32)
    qs = np.asarray(inputs["q_scale"], np.float32)
    ks = np.asarray(inputs["k_scale"], np.float32)
    ft = np.asarray(inputs["freqs_t"], np.float32)
    fh = np.asarray(inputs["freqs_h"], np.float32)
    fw = np.asarray(inputs["freqs_w"], np.float32)

    cos = np.zeros((N, HD // 2), np.float32)
    sin = np.zeros((N, HD // 2), np.float32)
    idx = np.arange(N)
    f_idx, h_idx, w_idx = idx // (Hg * Wg), (idx // Wg) % Hg, idx % Wg
    cos[:, 0:22], sin[:, 0:22] = ft[f_idx, :, 0], ft[f_idx, :, 1]
    cos[:, 22:43], sin[:, 22:43] = fh[h_idx, :, 0], fh[h_idx, :, 1]
    cos[:, 43:64], sin[:, 43:64] = fw[w_idx, :, 0], fw[w_idx, :, 1]
    C = np.repeat(cos, 2, axis=1).T.copy()               # [128, N]
    S = np.repeat(sin, 2, axis=1).T.copy()
    qs_sw = qs.reshape(64, 2)[:, ::-1].reshape(128)
    ks_sw = ks.reshape(64, 2)[:, ::-1].reshape(128)
    Cq, Sq = C * qs[:, None], S * qs_sw[:, None]
    Ck, Sk = C * ks[:, None], S * ks_sw[:, None]

    rotm = np.zeros((128, 128), np.float32)
    pr = np.arange(64)
    rotm[2 * pr + 1, 2 * pr] = -1.0
    rotm[2 * pr, 2 * pr + 1] = 1.0
    eye = np.eye(128, dtype=np.float32)
    ones2d = np.ones((128, 128), np.float32)

    xT = np.ascontiguousarray(x.T)                       # [D, N]
    perm_swap = np.concatenate([np.arange(1600, N), np.arange(0, 1600)])

    in_maps = []
    for core in range(8):
        pair, parity = core // 2, core % 2
        own, sh = 3 * pair + parity, 3 * pair + 2
        if parity == 0:
            xTc, Cqc, Sqc, Ckc, Skc = xT, Cq, Sq, Ck, Sk
        else:
            xTc = np.ascontiguousarray(xT[:, perm_swap])
            Cqc = np.ascontiguousarray(Cq[:, perm_swap])
            Sqc = np.ascontiguousarray(Sq[:, perm_swap])
            Ckc = np.ascontiguousarray(Ck[:, perm_swap])
            Skc = np.ascontiguousarray(Sk[:, perm_swap])
        w6 = np.stack([
            Wq[:, own * HD:(own + 1) * HD], Wq[:, sh * HD:(sh + 1) * HD],
            Wk[:, own * HD:(own + 1) * HD], Wk[:, sh * HD:(sh + 1) * HD],
            Wv[:, own * HD:(own + 1) * HD], Wv[:, sh * HD:(sh + 1) * HD],
        ])
        bias6 = np.stack([
            bq[own * HD:(own + 1) * HD], bq[sh * HD:(sh + 1) * HD],
            bk[own * HD:(own + 1) * HD], bk[sh * HD:(sh + 1) * HD],
            bv[own * HD:(own + 1) * HD], bv[sh * HD:(sh + 1) * HD],
        ], axis=1)
        wo2 = np.stack([
            Wo[own * HD:(own + 1) * HD, :], Wo[sh * HD:(sh + 1) * HD, :],
        ])
        in_maps.append({
            "xT": xTc.astype(bf16), "w6": np.ascontiguousarray(w6).astype(bf16),
            "bias6": np.ascontiguousarray(bias6),
            "cq": Cqc.astype(bf16), "sq": Sqc.astype(bf16),
            "ck": Ckc.astype(bf16), "sk": Skc.astype(bf16),
            "rotm": rotm.astype(bf16), "eye": eye.astype(bf16),
            "ones2d": ones2d.astype(bf16), "ones2df": ones2d, "eyef": eye,
            "wo2": np.ascontiguousarray(wo2).astype(bf16),
        })
    return in_maps, perm_swap


def _gather(results, perm_swap, bo):
    inv_swap = perm_swap  # swapping halves is its own inverse
    y = np.zeros((N, D), np.float32)
    for core in range(8):
        parity = core % 2
        yo = np.asarray(results[core]["y_own"], np.float32)
        ysh = np.asarray(results[core]["y_sh"], np.float32)
        if parity == 0:
            y += yo
            y[0:1600] += ysh
        else:
            y += yo[inv_swap]
            y[1600:3200] += ysh
    y += bo[None, :]
    return y[None]


def run_internal(inputs, trace=False, **kw):
    from concourse.bass_utils import run_bass_kernel_spmd

    nc = _get_nc()
    in_maps, perm_swap = _host_prep(inputs)
    res = run_bass_kernel_spmd(
        nc, in_maps, core_ids=list(range(8)), trace=trace, **kw
    )
    bo = np.asarray(inputs["bo"], np.float32)
    y = _gather(res.results, perm_swap, bo)
    return y, res


def kernel(**inputs):
    y, _ = run_internal(inputs, trace=False)
    return y


# revision 32
# speedup vs baseline: 1.4940x; 1.4940x over previous
"""Trainium2 Bass kernel for nn_MultiHeadAttention_61357902791348.

Sharding: 12 heads on 8 cores. Core pair (2p, 2p+1) owns heads {3p, 3p+1}
fully and splits head 3p+2's query rows (even core: rows [0,1600), odd:
[1600,3200)) -- balanced head/sequence-hybrid tensor parallelism with no
device collectives. Each core emits partial out-projection results; the
host sums the 8 partials and adds bo.

v2 rewrite vs baseline:
  * all PE operands bf16 (moving-stream bytes halved; fp32 PSUM accum)
  * no [1,W] single-lane vector/scalar ops: RMS-norm and softmax-sum
    rows are transposed onto partitions (K=1 ones-matmul trick) before
    rsqrt/reciprocal
  * softmax normalization deferred to the out-projection PSUM->SBUF copy
    as a per-partition activation scale (tokens on partitions there)
  * attention software-pipelined: scores(jt+1) issued before sm/ao(jt)
    so the exp latency on ACT hides under PE work
  * rope(q chunk) -> attention(chunk) -> out-proj(chunk) interleaved in
    one loop so DVE rope work hides under attention PE work
"""

import numpy as np

B, N, D = 1, 3200, 1536
NH, HD = 12, 128
F, Hg, Wg = 8, 20, 20
EPS = 1e-6
NS = 1600          # shared-head query rows per core
NCH = D // 128     # 12 D-chunks
PW = 400           # projection moving tile width
RW = 512           # rope / attention chunk width
NQ = N + NS        # 4800 q tokens per core (own + shared)
NK = 2 * N         # 6400 k tokens per core (own + shared heads)
NCHUNK = 38        # ceil(NQ / 128) 128-token chunks (last is 64 wide)

_CACHE = {}


def subtiles(total, width):
    return [(o, min(width, total - o)) for o in range(0, total, width)]


def _build():
    import concourse.bacc as bacc
    import concourse.mybir as mybir
    import concourse.tile as tile

    F32 = mybir.dt.float32
    BF16 = mybir.dt.bfloat16
    AF = mybir.ActivationFunctionType

    nc = bacc.Bacc("TRN2", target_bir_lowering=False, debug=False)

    xT = nc.dram_tensor("xT", [D, N], BF16, kind="ExternalInput")
    w6 = nc.dram_tensor("w6", [6, 128, NCH, HD], BF16, kind="ExternalInput")
    bias6 = nc.dram_tensor("bias6", [HD, 6], F32, kind="ExternalInput")
    cq = nc.dram_tensor("cq", [HD, N], BF16, kind="ExternalInput")
    sq = nc.dram_tensor("sq", [HD, N], BF16, kind="ExternalInput")
    ck = nc.dram_tensor("ck", [HD, N], BF16, kind="ExternalInput")
    sk = nc.dram_tensor("sk", [HD, N], BF16, kind="ExternalInput")
    rotm = nc.dram_tensor("rotm", [HD, HD], BF16, kind="ExternalInput")
    eye = nc.dram_tensor("eye", [HD, HD], BF16, kind="ExternalInput")
    ones2d = nc.dram_tensor("ones2d", [HD, HD], BF16, kind="ExternalInput")
    ones2df = nc.dram_tensor("ones2df", [HD, HD], F32, kind="ExternalInput")
    eyef = nc.dram_tensor("eyef", [HD, HD], F32, kind="ExternalInput")
    wo2 = nc.dram_tensor("wo2", [2, HD, D], BF16, kind="ExternalInput")
    y_own = nc.dram_tensor("y_own", [N, D], F32, kind="ExternalOutput")
    y_sh = nc.dram_tensor("y_sh", [NS, D], F32, kind="ExternalOutput")

    with tile.TileContext(nc) as tc:
        import contextlib

        stack = contextlib.ExitStack()
        with stack:
            persist = stack.enter_context(tc.tile_pool(name="persist", bufs=1))
            qT = persist.tile([128, NQ], BF16, tag="qT")
            kT = persist.tile([128, NK], BF16, tag="kT")
            vtok = persist.tile([128, 2, 25, HD], BF16, tag="vtok")
            aoT = persist.tile([128, NQ], BF16, tag="aoT")
            bias_sb = persist.tile([HD, 6], F32, tag="bias")
            nc.sync.dma_start(bias_sb[:], bias6[:])
            ones_sb = persist.tile([HD, HD], BF16, tag="ones")
            rot_sb = persist.tile([HD, HD], BF16, tag="rot")
            eye_sb = persist.tile([HD, HD], BF16, tag="eye")
            ones_f = persist.tile([HD, HD], F32, tag="onesf")
            eye_f = persist.tile([HD, HD], F32, tag="eyef")
            wo_sb = [persist.tile([HD, D], BF16, tag=f"wo{u}", name=f"wo_sb{u}")
                     for u in range(2)]
            tabs = {}
            for nm in ("cq", "sq", "ck", "sk"):
                tabs[nm] = persist.tile([128, N], BF16, tag=f"tab_{nm}",
                                        name=f"tab_{nm}")

            def load_aux():
                # issued after the first projection tiles so the Sync/DMA
                # queue prioritizes getting the PE started
                nc.sync.dma_start(ones_sb[:], ones2d[:])
                nc.sync.dma_start(rot_sb[:], rotm[:])
                nc.sync.dma_start(eye_sb[:], eye[:])
                nc.sync.dma_start(ones_f[:], ones2df[:])
                nc.sync.dma_start(eye_f[:], eyef[:])
                for u in range(2):
                    nc.sync.dma_start(wo_sb[u][:], wo2[u, :, :])
                for nm, dr in (("cq", cq), ("sq", sq), ("ck", ck), ("sk", sk)):
                    nc.sync.dma_start(tabs[nm][:], dr[:])
            rsk_sb = persist.tile([128, 50], F32, tag="rsk")
            inv_sb = persist.tile([128, NCHUNK], F32, tag="inv")
            rows_all = persist.tile([1, NCHUNK * 128], F32, tag="rows_all")
            rsq_all = persist.tile([128, NCHUNK], F32, tag="rsq_all")
            bias_q = persist.tile([128, 1], F32, tag="bias_q")
            nc.vector.memset(bias_q[:], HD * EPS)
            bias_k = persist.tile([128, 1], F32, tag="bias_k")
            nc.vector.memset(bias_k[:], EPS)

            row_ps = stack.enter_context(
                tc.tile_pool(name="row_ps", bufs=1, space="PSUM"))   # [1,512]
            smt_ps = stack.enter_context(
                tc.tile_pool(name="smt_ps", bufs=1, space="PSUM"))   # [128,8]
            rtmp = stack.enter_context(tc.tile_pool(name="rtmp", bufs=3))
            pools = {}

            def rms_rs(kind, src_row_psum, w, dst_f32, dst_col0, uid):
                """Transpose ssq row [1,w] onto partitions, rsqrt there."""
                ssq_row = rtmp.tile([1, RW], F32, tag="ssqr", name=f"sr{uid}")
                nc.scalar.activation(ssq_row[:, :w], src_row_psum, AF.Copy)
                smt = smt_ps.tile([128, 8], F32, tag="smt", name=f"smt{uid}")
                ncc = (w + 127) // 128
                for i in range(ncc):
                    lo = i * 128
                    ccw = min(128, w - lo)
                    nc.tensor.matmul(
                        smt[0:ccw, 2 * i:2 * i + 2],
                        ssq_row[0:1, lo:lo + ccw],
                        ones_f[0:1, 0:2],
                        start=True, stop=True,
                    )
                sq_t = rtmp.tile([128, 8], F32, tag="sqt", name=f"sq{uid}")
                if kind == "q":
                    nc.scalar.activation(
                        sq_t[:, :ncc], smt[:, 0:2 * ncc:2], AF.Sqrt,
                        bias=bias_q[:], scale=1.0,
                    )
                else:
                    nc.scalar.activation(
                        sq_t[:, :ncc], smt[:, 0:2 * ncc:2], AF.Sqrt,
                        bias=bias_k[:], scale=1.0 / HD,
                    )
                nc.vector.reciprocal(
                    dst_f32[:, dst_col0:dst_col0 + ncc], sq_t[:, :ncc])
                return ncc

            def rope_stats(kind, o, w, uid):
                """Sum-of-squares -> rs for one tile; q also flattens rs to
                a row in rows_all (for the later broadcast matmul)."""
                big = qT if kind == "q" else kT
                src = big[:, o:o + w]
                q2 = rtmp.tile([128, RW], BF16, tag="q2", name=f"q2{uid}")
                # SBUF-only op; Pool engine is idle before the chunk loop
                nc.gpsimd.tensor_mul(q2[:, :w], src, src)
                ssq = row_ps.tile([1, RW], F32, tag="row", name=f"ssq{uid}")
                nc.tensor.matmul(
                    ssq[:, :w], ones_sb[:, 0:1], q2[:, :w],
                    start=True, stop=True,
                )
                if kind == "k":
                    rms_rs("k", ssq[:, :w], w, rsk_sb, o // 128, uid)
                    return
                rms_rs("q", ssq[:, :w], w, rsq_all, o // 128, uid)

            def flatten_rs(o, w, uid):
                """rsq_all columns -> a row segment of rows_all."""
                g0 = o // 128
                ncc = (w + 127) // 128
                trp = pools["trp"].tile([128, 128], F32, tag="trp",
                                        name=f"trp{uid}")
                nc.tensor.transpose(
                    trp[0:ncc, :], rsq_all[:, g0:g0 + ncc], eye_f[:])
                rows8 = rtmp.tile([8, 128], F32, tag="rows8", name=f"r8{uid}")
                nc.vector.tensor_copy(rows8[0:ncc, :], trp[0:ncc, :])
                nc.sync.dma_start(
                    rows_all[0:1, o:o + ncc * 128], rows8[0:ncc, :])

            def rope_apply(kind, o, tok, w, uid, ps512):
                big = qT if kind == "q" else kT
                ctab = tabs["cq"] if kind == "q" else tabs["ck"]
                stab = tabs["sq"] if kind == "q" else tabs["sk"]
                src = big[:, o:o + w]
                rot = ps512.tile([128, RW], F32, tag="ps", name=f"rt{uid}")
                nc.tensor.matmul(
                    rot[:, :w], rot_sb[:], src, start=True, stop=True)
                if kind == "q":
                    bcp = ps512.tile([128, RW], F32, tag="ps", name=f"bc{uid}")
                    ncc = (w + 127) // 128
                    for i in range(ncc):
                        lo = i * 128
                        ccw = min(128, w - lo)
                        nc.tensor.matmul(
                            bcp[:, lo:lo + ccw], ones_f[0:1, :],
                            rows_all[0:1, o + lo:o + lo + ccw],
                            start=True, stop=True,
                        )
                m1 = rtmp.tile([128, RW], BF16, tag="m1", name=f"m1{uid}")
                if kind == "k":
                    nc.gpsimd.tensor_mul(m1[:, :w], src, ctab[:, tok:tok + w])
                else:
                    nc.vector.tensor_mul(m1[:, :w], src, ctab[:, tok:tok + w])
                m2 = rtmp.tile([128, RW], BF16, tag="m2", name=f"m2{uid}")
                nc.vector.tensor_mul(m2[:, :w], rot[:, :w], stab[:, tok:tok + w])
                if kind == "k":
                    nc.vector.tensor_add(src, m1[:, :w], m2[:, :w])
                else:
                    qr = rtmp.tile([128, RW], BF16, tag="qr", name=f"qr{uid}")
                    nc.gpsimd.tensor_add(qr[:, :w], m1[:, :w], m2[:, :w])
                    nc.vector.tensor_mul(src, qr[:, :w], bcp[:, :w])

            # ---------------- projection phase ----------------
            with tc.tile_pool(name="vt", bufs=1) as vt_pool:
                vT = vt_pool.tile([128, NK], BF16, tag="vT")
                with tc.tile_pool(name="xt", bufs=1) as xt_pool, \
                     tc.tile_pool(name="wld", bufs=3) as w_pool, \
                     tc.tile_pool(name="pp", bufs=5, space="PSUM") as pp:
                    wt_next = None
                    for half in range(2):
                        h0 = half * 1600
                        if half == 0:
                            # first weights ahead of the big x transfers so
                            # the PE starts as soon as x chunk 0 lands
                            wt_next = w_pool.tile([128, NCH, HD], BF16,
                                                  tag="w", name="wt_first")
                            nc.sync.dma_start(wt_next[:], w6[0, :, :, :])
                        xts = []
                        for c in range(NCH):
                            xt = xt_pool.tile([128, 1600], BF16, tag=f"xt{c}")
                            nc.sync.dma_start(
                                xt[:], xT[c * 128:(c + 1) * 128, h0:h0 + 1600])
                            xts.append(xt)
                        if half == 1:
                            # aux tables land during half-1 compute, well
                            # before rope/out-proj need them, without delaying
                            # any projection weight loads
                            load_aux()
                        # blocks: 0 q_own, 1 q_sh, 2 k_own, 3 k_sh, 4 v_own, 5 v_sh
                        for b in range(6):
                            if b == 1 and half == 1:
                                continue
                            if b == 0:
                                dst, d0 = qT, h0
                            elif b == 1:
                                dst, d0 = qT, N + h0
                            elif b in (2, 3):
                                dst, d0 = kT, (b - 2) * N + h0
                            else:
                                dst, d0 = vT, (b - 4) * N + h0
                            if wt_next is not None:
                                wt_all, wt_next = wt_next, None
                            else:
                                wt_all = w_pool.tile([128, NCH, HD], BF16,
                                                     tag="w",
                                                     name=f"wt{half}_{b}")
                                nc.sync.dma_start(wt_all[:], w6[b, :, :, :])
                            # c-outer with 4 parallel psum tiles: each
                            # stationary loads once per block (12 LDW instead
                            # of 48), keeping the PE at stream rate
                            tiles4 = subtiles(1600, PW)
                            pss = [pp.tile([128, PW], F32, tag="pp",
                                           name=f"pp{half}_{b}_{oi}")
                                   for oi in range(len(tiles4))]
                            for c in range(NCH):
                                for oi, (o, w) in enumerate(tiles4):
                                    nc.tensor.matmul(
                                        pss[oi][:, :w], wt_all[:, c, :],
                                        xts[c][:, o:o + w],
                                        start=(c == 0), stop=(c == NCH - 1),
                                    )
                            for oi, (o, w) in enumerate(tiles4):
                                nc.vector.tensor_scalar_add(
                                    dst[:, d0 + o:d0 + o + w], pss[oi][:, :w],
                                    bias_sb[:, b:b + 1],
                                )

                k_tiles = []
                for seg in range(2):
                    for (ol, w) in subtiles(N, RW):
                        k_tiles.append((seg * N + ol, ol, w))
                chunks_pre = []
                for (ol, w) in subtiles(N, RW):
                    chunks_pre.append((0, ol, ol, w))     # unit, qcol, tok, w
                for (ol, w) in subtiles(NS, RW):
                    chunks_pre.append((1, N + ol, ol, w))

                with tc.tile_pool(name="ps512", bufs=4, space="PSUM") as ps512:
                    # V transposes (5 per psum bank) interleaved with rope(k)
                    with tc.tile_pool(name="vtp", bufs=2, space="PSUM") as vtp:
                        for i in range(len(k_tiles)):
                            if i < 10:
                                h, g = divmod(i, 5)
                                tpg = vtp.tile([128, 5, HD], BF16, tag="tp",
                                               name=f"tp{i}")
                                for k5 in range(5):
                                    jt = g * 5 + k5
                                    nc.tensor.transpose(
                                        tpg[:, k5, :],
                                        vT[:, h * N + jt * 128:
                                           h * N + (jt + 1) * 128],
                                        eye_sb[:],
                                    )
                                nc.scalar.activation(
                                    vtok[:, h, g * 5:(g + 1) * 5, :], tpg[:],
                                    AF.Copy)
                            (o, tok, w) = k_tiles[i]
                            rope_stats("k", o, w, f"k{i}")
                            rope_apply("k", o, tok, w, f"k{i}", ps512)
                            if i < len(chunks_pre):
                                rope_stats("q", chunks_pre[i][1],
                                           chunks_pre[i][3], f"q{i}")

                    chunks = chunks_pre

                    # flatten per-chunk rs columns into rows_all segments;
                    # the transpose psum bank frees before aops opens
                    with tc.tile_pool(name="trp_ps", bufs=1,
                                      space="PSUM") as trp_ps:
                        pools["trp"] = trp_ps
                        for ci, (unit, gco, tok, cw) in enumerate(chunks):
                            flatten_rs(gco, cw, f"q{ci}")

                    with tc.tile_pool(name="aops", bufs=2, space="PSUM") as aops, \
                         tc.tile_pool(name="expp", bufs=6) as expp, \
                         tc.tile_pool(name="accp", bufs=2) as accp, \
                         tc.tile_pool(name="yout", bufs=3) as yout:

                        state = {}

                        def attention(ci, unit, gco, cw):
                            head = unit
                            ao = aops.tile([128, RW], F32, tag="ao",
                                           name=f"ao{ci}")
                            acc_a = accp.tile([128, RW], F32, tag="acca",
                                              name=f"acca{ci}")
                            acc_b = accp.tile([128, RW], F32, tag="accb",
                                              name=f"accb{ci}")
                            exs = {}
                            prev = None
                            for jt in range(25):
                                gjt = head * 25 + jt
                                sc = ps512.tile([128, RW], F32, tag="ps",
                                                name=f"sc{ci}_{jt}")
                                nc.tensor.matmul(
                                    sc[:, :cw],
                                    kT[:, gjt * 128:(gjt + 1) * 128],
                                    qT[:, gco:gco + cw],
                                    start=True, stop=True,
                                )
                                ex = expp.tile([128, RW], BF16, tag="ex",
                                               name=f"ex{ci}_{jt}")
                                nc.scalar.activation(
                                    ex[:, :cw], sc[:, :cw], AF.Exp,
                                    scale=rsk_sb[:, gjt:gjt + 1],
                                )
                                exs[jt] = ex
                                # softmax sums: two parallel f32 chains; Pool
                                # is slower per op so it gets the 3:2 share
                                # that equalizes finish times
                                if jt in (0, 1):
                                    pass  # consumed by the pair-starts below
                                elif jt == 2:
                                    nc.gpsimd.tensor_add(
                                        acc_a[:, :cw], exs[0][:, :cw],
                                        exs[2][:, :cw])
                                elif jt == 3:
                                    nc.vector.tensor_add(
                                        acc_b[:, :cw], exs[1][:, :cw],
                                        exs[3][:, :cw])
                                elif jt % 2 == 0:
                                    nc.gpsimd.tensor_add(
                                        acc_a[:, :cw], acc_a[:, :cw],
                                        ex[:, :cw])
                                else:
                                    nc.vector.tensor_add(
                                        acc_b[:, :cw], acc_b[:, :cw],
                                        ex[:, :cw])
                                if prev is not None:
                                    pex, pjt = prev
                                    nc.tensor.matmul(
                                        ao[:, :cw], vtok[:, head, pjt, :],
                                        pex[:, :cw],
                                        start=(pjt == 0), stop=False,
                                    )
                                prev = (ex, jt)
                            pex, pjt = prev
                            nc.tensor.matmul(
                                ao[:, :cw], vtok[:, head, pjt, :], pex[:, :cw],
                                start=False, stop=True,
                            )
                            state[ci] = (ao, acc_a, acc_b)

                        def tail(ci, unit, gco, tok, cw):
                            ao, acc_a, acc_b = state.pop(ci)
                            g0 = gco // 128
                            # merge the two f32 chains into bf16 (single final
                            # rounding) so the reduce matmul runs at 1 cyc/row
                            accm = accp.tile([128, RW], BF16, tag="accm",
                                             name=f"accm{ci}")
                            nc.gpsimd.tensor_add(
                                accm[:, :cw], acc_b[:, :cw], acc_a[:, :cw])
                            sm = row_ps.tile([1, RW], F32, tag="row",
                                             name=f"sm{ci}")
                            nc.tensor.matmul(
                                sm[:, :cw], ones_sb[:, 0:1], accm[:, :cw],
                                start=True, stop=True,
                            )
                            smrow = rtmp.tile([1, RW], F32, tag="ssqr",
                                              name=f"smr{ci}")
                            nc.vector.tensor_copy(smrow[:, :cw], sm[:, :cw])
                            smt = smt_ps.tile([128, 8], F32, tag="smt",
                                              name=f"smT{ci}")
                            ncc = (cw + 127) // 128
                            for i in range(ncc):
                                lo = i * 128
                                ccw = min(128, cw - lo)
                                nc.tensor.matmul(
                                    smt[0:ccw, 2 * i:2 * i + 2],
                                    smrow[0:1, lo:lo + ccw],
                                    ones_f[0:1, 0:2],
                                    start=True, stop=True,
                                )
                            nc.vector.reciprocal(
                                inv_sb[:, g0:g0 + ncc], smt[:, 0:2 * ncc:2])
                            nc.vector.tensor_copy(
                                aoT[:, gco:gco + cw], ao[:, :cw])

                        def outproj(ci, unit, gco, tok, cw):
                            ydst = y_own if unit == 0 else y_sh
                            for (it, iw) in subtiles(cw, 128):
                                git = gco + it
                                gidx = git // 128
                                yt = yout.tile([128, D], F32, tag="yt",
                                               name=f"yt{ci}_{it}")
                                for ct3 in range(3):
                                    op = ps512.tile([128, RW], F32, tag="ps",
                                                    name=f"op{ci}_{it}_{ct3}")
                                    nc.tensor.matmul(
                                        op[0:iw, :], aoT[:, git:git + iw],
                                        wo_sb[unit][:, ct3 * 512:(ct3 + 1) * 512],
                                        start=True, stop=True,
                                    )
                                    if not (ct3 == 1 and (it // 128) % 2 == 0):
                                        nc.vector.tensor_scalar_mul(
                                            yt[0:iw, ct3 * 512:(ct3 + 1) * 512],
                                            op[0:iw, :],
                                            inv_sb[0:iw, gidx:gidx + 1],
                                        )
                                    else:
                                        # Copy lives in every ACT table: no
                                        # act-table swap against Exp
                                        nc.scalar.activation(
                                            yt[0:iw, ct3 * 512:(ct3 + 1) * 512],
                                            op[0:iw, :], AF.Copy,
                                            scale=inv_sb[0:iw, gidx:gidx + 1],
                                        )
                                nc.sync.dma_start(
                                    ydst[tok + it:tok + it + iw, :], yt[0:iw, :])

                        # tail(ci-1) lands at the head of iteration ci so its
                        # psum drains finish under rope/attention PE work;
                        # outproj(ci-1) runs after attention(ci) when inv and
                        # aoT are long ready -- the PE never stalls on them
                        for ci, (unit, gco, tok, cw) in enumerate(chunks):
                            rope_apply("q", gco, tok, cw, f"q{ci}", ps512)
                            if ci > 0:
                                tail(ci - 1, *chunks[ci - 1])
                            attention(ci, unit, gco, cw)
                            if ci > 0:
                                outproj(ci - 1, *chunks[ci - 1])
                        tail(len(chunks) - 1, *chunks[-1])
                        outproj(len(chunks) - 1, *chunks[-1])

    nc.compile()
    return nc


def _get_nc():
    if "nc" not in _CACHE:
        _CACHE["nc"] = _build()
    return _CACHE["nc"]


def _host_prep(inputs):
    import ml_dtypes

    bf16 = ml_dtypes.bfloat16
    x = np.asarray(inputs["x"], np.float32)[0]          # [N, D]
    Wq = np.asarray(inputs["Wq"], np.float32)
    Wk = np.asarray(inputs["Wk"], np.float32)
    Wv = np.asarray(inputs["Wv"], np.float32)
    Wo = np.asarray(inputs["Wo"], np.float32)
    bq = np.asarray(inputs["bq"], np.float32)
    bk = np.asarray(inputs["bk"], np.float32)
    bv = np.asarray(inputs["bv"], np.float32)
    qs = np.asarray(inputs["q_scale"], np.float32)
    ks = np.asarray(inputs["k_scale"], np.float32)
    ft = np.asarray(inputs["freqs_t"], np.float32)
    fh = np.asarray(inputs["freqs_h"], np.float32)
    fw = np.asarray(inputs["freqs_w"], np.float32)

    cos = np.zeros((N, HD // 2), np.float32)
    sin = np.zeros((N, HD // 2), np.float32)
    idx = np.arange(N)
    f_idx, h_idx, w_idx = idx // (Hg * Wg), (idx // Wg) % Hg, idx % Wg
    cos[:, 0:22], sin[:, 0:22] = ft[f_idx, :, 0], ft[f_idx, :, 1]
    cos[:, 22:43], sin[:, 22:43] = fh[h_idx, :, 0], fh[h_idx, :, 1]
    cos[:, 43:64], sin[:, 43:64] = fw[w_idx, :, 0], fw[w_idx, :, 1]
    C = np.repeat(cos, 2, axis=1).T.copy()               # [128, N]
    S = np.repeat(sin, 2, axis=1).T.copy()
    qs_sw = qs.reshape(64, 2)[:, ::-1].reshape(128)
    ks_sw = ks.reshape(64, 2)[:, ::-1].reshape(128)
    Cq, Sq = C * qs[:, None], S * qs_sw[:, None]
    Ck, Sk = C * ks[:, None], S * ks_sw[:, None]

    rotm = np.zeros((128, 128), np.float32)
    pr = np.arange(64)
    rotm[2 * pr + 1, 2 * pr] = -1.0
    rotm[2 * pr, 2 * pr + 1] = 1.0
    eye = np.eye(128, dtype=np.float32)
    ones2d = np.ones((128, 128), np.float32)

    xT = np.ascontiguousarray(x.T)                       # [D, N]
    perm_swap = np.concatenate([np.arange(1600, N), np.arange(0, 1600)])

    in_maps = []
    for core in range(8):
        pair, parity = core // 2, core % 2
        own, sh = 3 * pair + parity, 3 * pair + 2
        if parity == 0:
            xTc, Cqc, Sqc, Ckc, Skc = xT, Cq, Sq, Ck, Sk
        else:
            xTc = np.ascontiguousarray(xT[:, perm_swap])
            Cqc = np.ascontiguousarray(Cq[:, perm_swap])
            Sqc = np.ascontiguousarray(Sq[:, perm_swap])
            Ckc = np.ascontiguousarray(Ck[:, perm_swap])
            Skc = np.ascontiguousarray(Sk[:, perm_swap])
        w6 = np.stack([
            Wq[:, own * HD:(own + 1) * HD], Wq[:, sh * HD:(sh + 1) * HD],
            Wk[:, own * HD:(own + 1) * HD], Wk[:, sh * HD:(sh + 1) * HD],
            Wv[:, own * HD:(own + 1) * HD], Wv[:, sh * HD:(sh + 1) * HD],
        ])
        # [6, D, HD] -> [6, 128, NCH, HD] so each block loads in ONE dma
        w6 = w6.reshape(6, NCH, 128, HD).transpose(0, 2, 1, 3)
        bias6 = np.stack([
            bq[own * HD:(own + 1) * HD], bq[sh * HD:(sh + 1) * HD],
            bk[own * HD:(own + 1) * HD], bk[sh * HD:(sh + 1) * HD],
            bv[own * HD:(own + 1) * HD], bv[sh * HD:(sh + 1) * HD],
        ], axis=1)
        wo2 = np.stack([
            Wo[own * HD:(own + 1) * HD, :], Wo[sh * HD:(sh + 1) * HD, :],
        ])
        in_maps.append({
            "xT": xTc.astype(bf16), "w6": np.ascontiguousarray(w6).astype(bf16),
            "bias6": np.ascontiguousarray(bias6),
            "cq": Cqc.astype(bf16), "sq": Sqc.astype(bf16),
            "ck": Ckc.astype(bf16), "sk": Skc.astype(bf16),
            "rotm": rotm.astype(bf16), "eye": eye.astype(bf16),
            "ones2d": ones2d.astype(bf16), "ones2df": ones2d, "eyef": eye,
            "wo2": np.ascontiguousarray(wo2).astype(bf16),
        })
    return in_maps, perm_swap


def _gather(results, perm_swap, bo):
    inv_swap = perm_swap  # swapping halves is its own inverse
    y = np.zeros((N, D), np.float32)
    for core in range(8):
        parity = core % 2
        yo = np.asarray(results[core]["y_own"], np.float32)
        ysh = np.asarray(results[core]["y_sh"], np.float32)
        if parity == 0:
            y += yo
            y[0:1600] += ysh
        else:
            y += yo[inv_swap]
            y[1600:3200] += ysh
    y += bo[None, :]
    return y[None]


def run_internal(inputs, trace=False, **kw):
    from concourse.bass_utils import run_bass_kernel_spmd

    nc = _get_nc()
    in_maps, perm_swap = _host_prep(inputs)
    res = run_bass_kernel_spmd(
        nc, in_maps, core_ids=list(range(8)), trace=trace, **kw
    )
    bo = np.asarray(inputs["bo"], np.float32)
    y = _gather(res.results, perm_swap, bo)
    return y, res


def kernel(**inputs):
    y, _ = run_internal(inputs, trace=False)
    return y


# revision 33
# speedup vs baseline: 1.5968x; 1.0688x over previous
"""Trainium2 Bass kernel for nn_MultiHeadAttention_61357902791348.

Sharding: 12 heads on 8 cores. Core pair (2p, 2p+1) owns heads {3p, 3p+1}
fully and splits head 3p+2's query rows (even core: rows [0,1600), odd:
[1600,3200)) -- balanced head/sequence-hybrid tensor parallelism with no
device collectives. Each core emits partial out-projection results; the
host sums the 8 partials and adds bo.

v2 rewrite vs baseline:
  * all PE operands bf16 (moving-stream bytes halved; fp32 PSUM accum)
  * no [1,W] single-lane vector/scalar ops: RMS-norm and softmax-sum
    rows are transposed onto partitions (K=1 ones-matmul trick) before
    rsqrt/reciprocal
  * softmax normalization deferred to the out-projection PSUM->SBUF copy
    as a per-partition activation scale (tokens on partitions there)
  * attention software-pipelined: scores(jt+1) issued before sm/ao(jt)
    so the exp latency on ACT hides under PE work
  * rope(q chunk) -> attention(chunk) -> out-proj(chunk) interleaved in
    one loop so DVE rope work hides under attention PE work
"""

import numpy as np

B, N, D = 1, 3200, 1536
NH, HD = 12, 128
F, Hg, Wg = 8, 20, 20
EPS = 1e-6
NS = 1600          # shared-head query rows per core
NCH = D // 128     # 12 D-chunks
PW = 400           # projection moving tile width
RW = 512           # rope / attention chunk width
NQ = N + NS        # 4800 q tokens per core (own + shared)
NK = 2 * N         # 6400 k tokens per core (own + shared heads)
NCHUNK = 38        # ceil(NQ / 128) 128-token chunks (last is 64 wide)

_CACHE = {}


def subtiles(total, width):
    return [(o, min(width, total - o)) for o in range(0, total, width)]


def _build():
    import concourse.bacc as bacc
    import concourse.mybir as mybir
    import concourse.tile as tile

    F32 = mybir.dt.float32
    BF16 = mybir.dt.bfloat16
    AF = mybir.ActivationFunctionType

    nc = bacc.Bacc("TRN2", target_bir_lowering=False, debug=False)

    xT = nc.dram_tensor("xT", [D, N], BF16, kind="ExternalInput")
    w6 = nc.dram_tensor("w6", [6, 128, NCH, HD], BF16, kind="ExternalInput")
    bias6 = nc.dram_tensor("bias6", [HD, 6], F32, kind="ExternalInput")
    cq = nc.dram_tensor("cq", [HD, N], BF16, kind="ExternalInput")
    sq = nc.dram_tensor("sq", [HD, N], BF16, kind="ExternalInput")
    ck = nc.dram_tensor("ck", [HD, N], BF16, kind="ExternalInput")
    sk = nc.dram_tensor("sk", [HD, N], BF16, kind="ExternalInput")
    rotm = nc.dram_tensor("rotm", [HD, HD], BF16, kind="ExternalInput")
    eye = nc.dram_tensor("eye", [HD, HD], BF16, kind="ExternalInput")
    ones2d = nc.dram_tensor("ones2d", [HD, HD], BF16, kind="ExternalInput")
    ones2df = nc.dram_tensor("ones2df", [HD, HD], F32, kind="ExternalInput")
    eyef = nc.dram_tensor("eyef", [HD, HD], F32, kind="ExternalInput")
    wo2 = nc.dram_tensor("wo2", [2, HD, D], BF16, kind="ExternalInput")
    y_own = nc.dram_tensor("y_own", [N, D], F32, kind="ExternalOutput")
    y_sh = nc.dram_tensor("y_sh", [NS, D], F32, kind="ExternalOutput")

    with tile.TileContext(nc) as tc:
        import contextlib

        stack = contextlib.ExitStack()
        with stack:
            persist = stack.enter_context(tc.tile_pool(name="persist", bufs=1))
            qT = persist.tile([128, NQ], BF16, tag="qT")
            kT = persist.tile([128, NK], BF16, tag="kT")
            vtok = persist.tile([128, 2, 25, HD], BF16, tag="vtok")
            aoT = persist.tile([128, NQ], BF16, tag="aoT")
            bias_sb = persist.tile([HD, 6], F32, tag="bias")
            nc.sync.dma_start(bias_sb[:], bias6[:])
            ones_sb = persist.tile([HD, HD], BF16, tag="ones")
            rot_sb = persist.tile([HD, HD], BF16, tag="rot")
            eye_sb = persist.tile([HD, HD], BF16, tag="eye")
            ones_f = persist.tile([HD, HD], F32, tag="onesf")
            eye_f = persist.tile([HD, HD], F32, tag="eyef")
            wo_sb = [persist.tile([HD, D], BF16, tag=f"wo{u}", name=f"wo_sb{u}")
                     for u in range(2)]
            tabs = {}
            for nm in ("cq", "sq", "ck", "sk"):
                tabs[nm] = persist.tile([128, N], BF16, tag=f"tab_{nm}",
                                        name=f"tab_{nm}")

            def load_aux():
                # issued after the first projection tiles so the Sync/DMA
                # queue prioritizes getting the PE started
                nc.sync.dma_start(ones_sb[:], ones2d[:])
                nc.sync.dma_start(rot_sb[:], rotm[:])
                nc.sync.dma_start(eye_sb[:], eye[:])
                nc.sync.dma_start(ones_f[:], ones2df[:])
                nc.sync.dma_start(eye_f[:], eyef[:])
                for u in range(2):
                    nc.sync.dma_start(wo_sb[u][:], wo2[u, :, :])
                for nm, dr in (("cq", cq), ("sq", sq), ("ck", ck), ("sk", sk)):
                    nc.sync.dma_start(tabs[nm][:], dr[:])
            rsk_sb = persist.tile([128, 50], F32, tag="rsk")
            inv_sb = persist.tile([128, NCHUNK], F32, tag="inv")
            rows_all = persist.tile([1, NCHUNK * 128], F32, tag="rows_all")
            rsq_all = persist.tile([128, NCHUNK], F32, tag="rsq_all")
            bias_q = persist.tile([128, 1], F32, tag="bias_q")
            nc.vector.memset(bias_q[:], HD * EPS)
            bias_k = persist.tile([128, 1], F32, tag="bias_k")
            nc.vector.memset(bias_k[:], EPS)

            row_ps = stack.enter_context(
                tc.tile_pool(name="row_ps", bufs=1, space="PSUM"))   # [1,512]
            smt_ps = stack.enter_context(
                tc.tile_pool(name="smt_ps", bufs=1, space="PSUM"))   # [128,8]
            rtmp = stack.enter_context(tc.tile_pool(name="rtmp", bufs=3))
            pools = {}

            def rms_rs(kind, src_row_psum, w, dst_f32, dst_col0, uid):
                """Transpose ssq row [1,w] onto partitions, rsqrt there."""
                ssq_row = rtmp.tile([1, RW], F32, tag="ssqr", name=f"sr{uid}")
                nc.scalar.activation(ssq_row[:, :w], src_row_psum, AF.Copy)
                smt = smt_ps.tile([128, 8], F32, tag="smt", name=f"smt{uid}")
                ncc = (w + 127) // 128
                for i in range(ncc):
                    lo = i * 128
                    ccw = min(128, w - lo)
                    nc.tensor.matmul(
                        smt[0:ccw, 2 * i:2 * i + 2],
                        ssq_row[0:1, lo:lo + ccw],
                        ones_f[0:1, 0:2],
                        start=True, stop=True,
                    )
                sq_t = rtmp.tile([128, 8], F32, tag="sqt", name=f"sq{uid}")
                if kind == "q":
                    nc.scalar.activation(
                        sq_t[:, :ncc], smt[:, 0:2 * ncc:2], AF.Sqrt,
                        bias=bias_q[:], scale=1.0,
                    )
                else:
                    nc.scalar.activation(
                        sq_t[:, :ncc], smt[:, 0:2 * ncc:2], AF.Sqrt,
                        bias=bias_k[:], scale=1.0 / HD,
                    )
                nc.vector.reciprocal(
                    dst_f32[:, dst_col0:dst_col0 + ncc], sq_t[:, :ncc])
                return ncc

            def rope_stats(kind, o, w, uid):
                """Sum-of-squares -> rs for one tile; q also flattens rs to
                a row in rows_all (for the later broadcast matmul)."""
                big = qT if kind == "q" else kT
                src = big[:, o:o + w]
                q2 = rtmp.tile([128, RW], BF16, tag="q2", name=f"q2{uid}")
                # SBUF-only op; Pool engine is idle before the chunk loop
                nc.gpsimd.tensor_mul(q2[:, :w], src, src)
                ssq = row_ps.tile([1, RW], F32, tag="row", name=f"ssq{uid}")
                nc.tensor.matmul(
                    ssq[:, :w], ones_sb[:, 0:1], q2[:, :w],
                    start=True, stop=True,
                )
                if kind == "k":
                    rms_rs("k", ssq[:, :w], w, rsk_sb, o // 128, uid)
                    return
                rms_rs("q", ssq[:, :w], w, rsq_all, o // 128, uid)

            def flatten_rs(o, w, uid):
                """rsq_all columns -> a row segment of rows_all."""
                g0 = o // 128
                ncc = (w + 127) // 128
                trp = pools["trp"].tile([128, 128], F32, tag="trp",
                                        name=f"trp{uid}")
                nc.tensor.transpose(
                    trp[0:ncc, :], rsq_all[:, g0:g0 + ncc], eye_f[:])
                rows8 = rtmp.tile([8, 128], F32, tag="rows8", name=f"r8{uid}")
                nc.vector.tensor_copy(rows8[0:ncc, :], trp[0:ncc, :])
                nc.sync.dma_start(
                    rows_all[0:1, o:o + ncc * 128], rows8[0:ncc, :])

            def rope_apply(kind, o, tok, w, uid, ps512):
                big = qT if kind == "q" else kT
                ctab = tabs["cq"] if kind == "q" else tabs["ck"]
                stab = tabs["sq"] if kind == "q" else tabs["sk"]
                src = big[:, o:o + w]
                rot = ps512.tile([128, RW], F32, tag="ps", name=f"rt{uid}")
                nc.tensor.matmul(
                    rot[:, :w], rot_sb[:], src, start=True, stop=True)
                if kind == "q":
                    bcp = ps512.tile([128, RW], F32, tag="ps", name=f"bc{uid}")
                    ncc = (w + 127) // 128
                    for i in range(ncc):
                        lo = i * 128
                        ccw = min(128, w - lo)
                        nc.tensor.matmul(
                            bcp[:, lo:lo + ccw], ones_f[0:1, :],
                            rows_all[0:1, o + lo:o + lo + ccw],
                            start=True, stop=True,
                        )
                m1 = rtmp.tile([128, RW], BF16, tag="m1", name=f"m1{uid}")
                if kind == "k":
                    nc.gpsimd.tensor_mul(m1[:, :w], src, ctab[:, tok:tok + w])
                else:
                    nc.vector.tensor_mul(m1[:, :w], src, ctab[:, tok:tok + w])
                m2 = rtmp.tile([128, RW], BF16, tag="m2", name=f"m2{uid}")
                nc.vector.tensor_mul(m2[:, :w], rot[:, :w], stab[:, tok:tok + w])
                if kind == "k":
                    nc.vector.tensor_add(src, m1[:, :w], m2[:, :w])
                else:
                    qr = rtmp.tile([128, RW], BF16, tag="qr", name=f"qr{uid}")
                    nc.vector.tensor_add(qr[:, :w], m1[:, :w], m2[:, :w])
                    nc.vector.tensor_mul(src, qr[:, :w], bcp[:, :w])

            # ---------------- projection phase ----------------
            with tc.tile_pool(name="vt", bufs=1) as vt_pool:
                vT = vt_pool.tile([128, NK], BF16, tag="vT")
                with tc.tile_pool(name="xt", bufs=1) as xt_pool, \
                     tc.tile_pool(name="wld", bufs=3) as w_pool, \
                     tc.tile_pool(name="pp", bufs=5, space="PSUM") as pp:
                    wt_next = None
                    for half in range(2):
                        h0 = half * 1600
                        if half == 0:
                            # first weights ahead of the big x transfers so
                            # the PE starts as soon as x chunk 0 lands
                            wt_next = w_pool.tile([128, NCH, HD], BF16,
                                                  tag="w", name="wt_first")
                            nc.sync.dma_start(wt_next[:], w6[0, :, :, :])
                        xts = []
                        for c in range(NCH):
                            xt = xt_pool.tile([128, 1600], BF16, tag=f"xt{c}")
                            nc.sync.dma_start(
                                xt[:], xT[c * 128:(c + 1) * 128, h0:h0 + 1600])
                            xts.append(xt)
                        if half == 1:
                            # aux tables land during half-1 compute, well
                            # before rope/out-proj need them, without delaying
                            # any projection weight loads
                            load_aux()
                        # blocks: 0 q_own, 1 q_sh, 2 k_own, 3 k_sh, 4 v_own, 5 v_sh
                        for b in range(6):
                            if b == 1 and half == 1:
                                continue
                            if b == 0:
                                dst, d0 = qT, h0
                            elif b == 1:
                                dst, d0 = qT, N + h0
                            elif b in (2, 3):
                                dst, d0 = kT, (b - 2) * N + h0
                            else:
                                dst, d0 = vT, (b - 4) * N + h0
                            if wt_next is not None:
                                wt_all, wt_next = wt_next, None
                            else:
                                wt_all = w_pool.tile([128, NCH, HD], BF16,
                                                     tag="w",
                                                     name=f"wt{half}_{b}")
                                nc.sync.dma_start(wt_all[:], w6[b, :, :, :])
                            # c-outer with 4 parallel psum tiles: each
                            # stationary loads once per block (12 LDW instead
                            # of 48), keeping the PE at stream rate
                            tiles4 = subtiles(1600, PW)
                            pss = [pp.tile([128, PW], F32, tag="pp",
                                           name=f"pp{half}_{b}_{oi}")
                                   for oi in range(len(tiles4))]
                            for c in range(NCH):
                                for oi, (o, w) in enumerate(tiles4):
                                    nc.tensor.matmul(
                                        pss[oi][:, :w], wt_all[:, c, :],
                                        xts[c][:, o:o + w],
                                        start=(c == 0), stop=(c == NCH - 1),
                                    )
                            for oi, (o, w) in enumerate(tiles4):
                                nc.vector.tensor_scalar_add(
                                    dst[:, d0 + o:d0 + o + w], pss[oi][:, :w],
                                    bias_sb[:, b:b + 1],
                                )

                k_tiles = []
                for seg in range(2):
                    for (ol, w) in subtiles(N, RW):
                        k_tiles.append((seg * N + ol, ol, w))
                chunks_pre = []
                for (ol, w) in subtiles(N, RW):
                    chunks_pre.append((0, ol, ol, w))     # unit, qcol, tok, w
                for (ol, w) in subtiles(NS, RW):
                    chunks_pre.append((1, N + ol, ol, w))

                with tc.tile_pool(name="ps512", bufs=4, space="PSUM") as ps512:
                    # V transposes (5 per psum bank) interleaved with rope(k)
                    with tc.tile_pool(name="vtp", bufs=2, space="PSUM") as vtp:
                        for i in range(len(k_tiles)):
                            if i < 10:
                                h, g = divmod(i, 5)
                                tpg = vtp.tile([128, 5, HD], BF16, tag="tp",
                                               name=f"tp{i}")
                                for k5 in range(5):
                                    jt = g * 5 + k5
                                    nc.tensor.transpose(
                                        tpg[:, k5, :],
                                        vT[:, h * N + jt * 128:
                                           h * N + (jt + 1) * 128],
                                        eye_sb[:],
                                    )
                                nc.scalar.activation(
                                    vtok[:, h, g * 5:(g + 1) * 5, :], tpg[:],
                                    AF.Copy)
                            (o, tok, w) = k_tiles[i]
                            rope_stats("k", o, w, f"k{i}")
                            rope_apply("k", o, tok, w, f"k{i}", ps512)
                            if i < len(chunks_pre):
                                rope_stats("q", chunks_pre[i][1],
                                           chunks_pre[i][3], f"q{i}")

                    chunks = chunks_pre

                    # flatten per-chunk rs columns into rows_all segments;
                    # the transpose psum bank frees before aops opens
                    with tc.tile_pool(name="trp_ps", bufs=1,
                                      space="PSUM") as trp_ps:
                        pools["trp"] = trp_ps
                        for ci, (unit, gco, tok, cw) in enumerate(chunks):
                            flatten_rs(gco, cw, f"q{ci}")

                    with tc.tile_pool(name="aops", bufs=2, space="PSUM") as aops, \
                         tc.tile_pool(name="expp", bufs=6) as expp, \
                         tc.tile_pool(name="accp", bufs=2) as accp, \
                         tc.tile_pool(name="yout", bufs=3) as yout:

                        state = {}

                        def attention(ci, unit, gco, cw):
                            head = unit
                            ao = aops.tile([128, RW], F32, tag="ao",
                                           name=f"ao{ci}")
                            acc_a = accp.tile([128, RW], F32, tag="acca",
                                              name=f"acca{ci}")
                            acc_b = accp.tile([128, RW], F32, tag="accb",
                                              name=f"accb{ci}")
                            exs = {}
                            prev = None
                            for jt in range(25):
                                gjt = head * 25 + jt
                                sc = ps512.tile([128, RW], F32, tag="ps",
                                                name=f"sc{ci}_{jt}")
                                nc.tensor.matmul(
                                    sc[:, :cw],
                                    kT[:, gjt * 128:(gjt + 1) * 128],
                                    qT[:, gco:gco + cw],
                                    start=True, stop=True,
                                )
                                ex = expp.tile([128, RW], BF16, tag="ex",
                                               name=f"ex{ci}_{jt}")
                                nc.scalar.activation(
                                    ex[:, :cw], sc[:, :cw], AF.Exp,
                                    scale=rsk_sb[:, gjt:gjt + 1],
                                )
                                exs[jt] = ex
                                # softmax sums: two parallel f32 chains; Pool
                                # is slower per op so it gets the 3:2 share
                                # that equalizes finish times
                                if jt in (0, 1):
                                    pass  # consumed by the pair-starts below
                                elif jt == 2:
                                    nc.gpsimd.tensor_add(
                                        acc_a[:, :cw], exs[0][:, :cw],
                                        exs[2][:, :cw])
                                elif jt == 3:
                                    nc.vector.tensor_add(
                                        acc_b[:, :cw], exs[1][:, :cw],
                                        exs[3][:, :cw])
                                elif jt % 2 == 0:
                                    nc.gpsimd.tensor_add(
                                        acc_a[:, :cw], acc_a[:, :cw],
                                        ex[:, :cw])
                                else:
                                    nc.vector.tensor_add(
                                        acc_b[:, :cw], acc_b[:, :cw],
                                        ex[:, :cw])
                                if prev is not None:
                                    pex, pjt = prev
                                    nc.tensor.matmul(
                                        ao[:, :cw], vtok[:, head, pjt, :],
                                        pex[:, :cw],
                                        start=(pjt == 0), stop=False,
                                    )
                                prev = (ex, jt)
                            pex, pjt = prev
                            nc.tensor.matmul(
                                ao[:, :cw], vtok[:, head, pjt, :], pex[:, :cw],
                                start=False, stop=True,
                            )
                            state[ci] = (ao, acc_a, acc_b)

                        def tail(ci, unit, gco, tok, cw):
                            ao, acc_a, acc_b = state.pop(ci)
                            g0 = gco // 128
                            # merge the two f32 chains into bf16 (single final
                            # rounding) so the reduce matmul runs at 1 cyc/row
                            accm = accp.tile([128, RW], BF16, tag="accm",
                                             name=f"accm{ci}")
                            nc.vector.tensor_add(
                                accm[:, :cw], acc_b[:, :cw], acc_a[:, :cw])
                            sm = row_ps.tile([1, RW], F32, tag="row",
                                             name=f"sm{ci}")
                            nc.tensor.matmul(
                                sm[:, :cw], ones_sb[:, 0:1], accm[:, :cw],
                                start=True, stop=True,
                            )
                            smrow = rtmp.tile([1, RW], F32, tag="ssqr",
                                              name=f"smr{ci}")
                            nc.vector.tensor_copy(smrow[:, :cw], sm[:, :cw])
                            smt = smt_ps.tile([128, 8], F32, tag="smt",
                                              name=f"smT{ci}")
                            ncc = (cw + 127) // 128
                            for i in range(ncc):
                                lo = i * 128
                                ccw = min(128, cw - lo)
                                nc.tensor.matmul(
                                    smt[0:ccw, 2 * i:2 * i + 2],
                                    smrow[0:1, lo:lo + ccw],
                                    ones_f[0:1, 0:2],
                                    start=True, stop=True,
                                )
                            nc.vector.reciprocal(
                                inv_sb[:, g0:g0 + ncc], smt[:, 0:2 * ncc:2])
                            nc.vector.tensor_copy(
                                aoT[:, gco:gco + cw], ao[:, :cw])

                        def outproj(ci, unit, gco, tok, cw):
                            ydst = y_own if unit == 0 else y_sh
                            for (it, iw) in subtiles(cw, 128):
                                git = gco + it
                                gidx = git // 128
                                yt = yout.tile([128, D], F32, tag="yt",
                                               name=f"yt{ci}_{it}")
                                for ct3 in range(3):
                                    op = ps512.tile([128, RW], F32, tag="ps",
                                                    name=f"op{ci}_{it}_{ct3}")
                                    nc.tensor.matmul(
                                        op[0:iw, :], aoT[:, git:git + iw],
                                        wo_sb[unit][:, ct3 * 512:(ct3 + 1) * 512],
                                        start=True, stop=True,
                                    )
                                    if (ct3 + it // 128) % 2 == 0:
                                        nc.vector.tensor_scalar_mul(
                                            yt[0:iw, ct3 * 512:(ct3 + 1) * 512],
                                            op[0:iw, :],
                                            inv_sb[0:iw, gidx:gidx + 1],
                                        )
                                    else:
                                        # Copy lives in every ACT table: no
                                        # act-table swap against Exp
                                        nc.scalar.activation(
                                            yt[0:iw, ct3 * 512:(ct3 + 1) * 512],
                                            op[0:iw, :], AF.Copy,
                                            scale=inv_sb[0:iw, gidx:gidx + 1],
                                        )
                                nc.sync.dma_start(
                                    ydst[tok + it:tok + it + iw, :], yt[0:iw, :])

                        # tail(ci-1) lands at the head of iteration ci so its
                        # psum drains finish under rope/attention PE work;
                        # outproj(ci-1) runs after attention(ci) when inv and
                        # aoT are long ready -- the PE never stalls on them
                        for ci, (unit, gco, tok, cw) in enumerate(chunks):
                            rope_apply("q", gco, tok, cw, f"q{ci}", ps512)
                            if ci > 0:
                                tail(ci - 1, *chunks[ci - 1])
                            attention(ci, unit, gco, cw)
                            if ci > 0:
                                outproj(ci - 1, *chunks[ci - 1])
                        tail(len(chunks) - 1, *chunks[-1])
                        outproj(len(chunks) - 1, *chunks[-1])

    nc.compile()
    return nc


def _get_nc():
    if "nc" not in _CACHE:
        _CACHE["nc"] = _build()
    return _CACHE["nc"]


def _host_prep(inputs):
    import ml_dtypes

    bf16 = ml_dtypes.bfloat16
    x = np.asarray(inputs["x"], np.float32)[0]          # [N, D]
    Wq = np.asarray(inputs["Wq"], np.float32)
    Wk = np.asarray(inputs["Wk"], np.float32)
    Wv = np.asarray(inputs["Wv"], np.float32)
    Wo = np.asarray(inputs["Wo"], np.float32)
    bq = np.asarray(inputs["bq"], np.float32)
    bk = np.asarray(inputs["bk"], np.float32)
    bv = np.asarray(inputs["bv"], np.float32)
    qs = np.asarray(inputs["q_scale"], np.float32)
    ks = np.asarray(inputs["k_scale"], np.float32)
    ft = np.asarray(inputs["freqs_t"], np.float32)
    fh = np.asarray(inputs["freqs_h"], np.float32)
    fw = np.asarray(inputs["freqs_w"], np.float32)

    cos = np.zeros((N, HD // 2), np.float32)
    sin = np.zeros((N, HD // 2), np.float32)
    idx = np.arange(N)
    f_idx, h_idx, w_idx = idx // (Hg * Wg), (idx // Wg) % Hg, idx % Wg
    cos[:, 0:22], sin[:, 0:22] = ft[f_idx, :, 0], ft[f_idx, :, 1]
    cos[:, 22:43], sin[:, 22:43] = fh[h_idx, :, 0], fh[h_idx, :, 1]
    cos[:, 43:64], sin[:, 43:64] = fw[w_idx, :, 0], fw[w_idx, :, 1]
    C = np.repeat(cos, 2, axis=1).T.copy()               # [128, N]
    S = np.repeat(sin, 2, axis=1).T.copy()
    qs_sw = qs.reshape(64, 2)[:, ::-1].reshape(128)
    ks_sw = ks.reshape(64, 2)[:, ::-1].reshape(128)
    Cq, Sq = C * qs[:, None], S * qs_sw[:, None]
    Ck, Sk = C * ks[:, None], S * ks_sw[:, None]

    rotm = np.zeros((128, 128), np.float32)
    pr = np.arange(64)
    rotm[2 * pr + 1, 2 * pr] = -1.0
    rotm[2 * pr, 2 * pr + 1] = 1.0
    eye = np.eye(128, dtype=np.float32)
    ones2d = np.ones((128, 128), np.float32)

    xT = np.ascontiguousarray(x.T)                       # [D, N]
    perm_swap = np.concatenate([np.arange(1600, N), np.arange(0, 1600)])

    in_maps = []
    for core in range(8):
        pair, parity = core // 2, core % 2
        own, sh = 3 * pair + parity, 3 * pair + 2
        if parity == 0:
            xTc, Cqc, Sqc, Ckc, Skc = xT, Cq, Sq, Ck, Sk
        else:
            xTc = np.ascontiguousarray(xT[:, perm_swap])
            Cqc = np.ascontiguousarray(Cq[:, perm_swap])
            Sqc = np.ascontiguousarray(Sq[:, perm_swap])
            Ckc = np.ascontiguousarray(Ck[:, perm_swap])
            Skc = np.ascontiguousarray(Sk[:, perm_swap])
        w6 = np.stack([
            Wq[:, own * HD:(own + 1) * HD], Wq[:, sh * HD:(sh + 1) * HD],
            Wk[:, own * HD:(own + 1) * HD], Wk[:, sh * HD:(sh + 1) * HD],
            Wv[:, own * HD:(own + 1) * HD], Wv[:, sh * HD:(sh + 1) * HD],
        ])
        # [6, D, HD] -> [6, 128, NCH, HD] so each block loads in ONE dma
        w6 = w6.reshape(6, NCH, 128, HD).transpose(0, 2, 1, 3)
        bias6 = np.stack([
            bq[own * HD:(own + 1) * HD], bq[sh * HD:(sh + 1) * HD],
            bk[own * HD:(own + 1) * HD], bk[sh * HD:(sh + 1) * HD],
            bv[own * HD:(own + 1) * HD], bv[sh * HD:(sh + 1) * HD],
        ], axis=1)
        wo2 = np.stack([
            Wo[own * HD:(own + 1) * HD, :], Wo[sh * HD:(sh + 1) * HD, :],
        ])
        in_maps.append({
            "xT": xTc.astype(bf16), "w6": np.ascontiguousarray(w6).astype(bf16),
            "bias6": np.ascontiguousarray(bias6),
            "cq": Cqc.astype(bf16), "sq": Sqc.astype(bf16),
            "ck": Ckc.astype(bf16), "sk": Skc.astype(bf16),
            "rotm": rotm.astype(bf16), "eye": eye.astype(bf16),
            "ones2d": ones2d.astype(bf16), "ones2df": ones2d, "eyef": eye,
            "wo2": np.ascontiguousarray(wo2).astype(bf16),
        })
    return in_maps, perm_swap


def _gather(results, perm_swap, bo):
    inv_swap = perm_swap  # swapping halves is its own inverse
    y = np.zeros((N, D), np.float32)
    for core in range(8):
        parity = core % 2
        yo = np.asarray(results[core]["y_own"], np.float32)
        ysh = np.asarray(results[core]["y_sh"], np.float32)
        if parity == 0:
            y += yo
            y[0:1600] += ysh
        else:
            y += yo[inv_swap]
            y[1600:3200] += ysh
    y += bo[None, :]
    return y[None]


def run_internal(inputs, trace=False, **kw):
    from concourse.bass_utils import run_bass_kernel_spmd

    nc = _get_nc()
    in_maps, perm_swap = _host_prep(inputs)
    res = run_bass_kernel_spmd(
        nc, in_maps, core_ids=list(range(8)), trace=trace, **kw
    )
    bo = np.asarray(inputs["bo"], np.float32)
    y = _gather(res.results, perm_swap, bo)
    return y, res


def kernel(**inputs):
    y, _ = run_internal(inputs, trace=False)
    return y


# revision 34
# speedup vs baseline: 1.6801x; 1.0521x over previous
"""Trainium2 Bass kernel for nn_MultiHeadAttention_61357902791348.

Sharding: 12 heads on 8 cores. Core pair (2p, 2p+1) owns heads {3p, 3p+1}
fully and splits head 3p+2's query rows (even core: rows [0,1600), odd:
[1600,3200)) -- balanced head/sequence-hybrid tensor parallelism with no
device collectives. Each core emits partial out-projection results; the
host sums the 8 partials and adds bo.

v2 rewrite vs baseline:
  * all PE operands bf16 (moving-stream bytes halved; fp32 PSUM accum)
  * no [1,W] single-lane vector/scalar ops: RMS-norm and softmax-sum
    rows are transposed onto partitions (K=1 ones-matmul trick) before
    rsqrt/reciprocal
  * softmax normalization deferred to the out-projection PSUM->SBUF copy
    as a per-partition activation scale (tokens on partitions there)
  * attention software-pipelined: scores(jt+1) issued before sm/ao(jt)
    so the exp latency on ACT hides under PE work
  * rope(q chunk) -> attention(chunk) -> out-proj(chunk) interleaved in
    one loop so DVE rope work hides under attention PE work
"""

import numpy as np

B, N, D = 1, 3200, 1536
NH, HD = 12, 128
F, Hg, Wg = 8, 20, 20
EPS = 1e-6
NS = 1600          # shared-head query rows per core
NCH = D // 128     # 12 D-chunks
PW = 400           # projection moving tile width
RW = 512           # rope / attention chunk width
NQ = N + NS        # 4800 q tokens per core (own + shared)
NK = 2 * N         # 6400 k tokens per core (own + shared heads)
NCHUNK = 38        # ceil(NQ / 128) 128-token chunks (last is 64 wide)

_CACHE = {}


def subtiles(total, width):
    return [(o, min(width, total - o)) for o in range(0, total, width)]


def _build():
    import concourse.bacc as bacc
    import concourse.mybir as mybir
    import concourse.tile as tile

    F32 = mybir.dt.float32
    BF16 = mybir.dt.bfloat16
    AF = mybir.ActivationFunctionType

    nc = bacc.Bacc("TRN2", target_bir_lowering=False, debug=False)

    xT = nc.dram_tensor("xT", [D, N], BF16, kind="ExternalInput")
    w6 = nc.dram_tensor("w6", [6, 128, NCH, HD], BF16, kind="ExternalInput")
    bias6 = nc.dram_tensor("bias6", [HD, 6], F32, kind="ExternalInput")
    cq = nc.dram_tensor("cq", [HD, N], BF16, kind="ExternalInput")
    sq = nc.dram_tensor("sq", [HD, N], BF16, kind="ExternalInput")
    ck = nc.dram_tensor("ck", [HD, N], BF16, kind="ExternalInput")
    sk = nc.dram_tensor("sk", [HD, N], BF16, kind="ExternalInput")
    rotm = nc.dram_tensor("rotm", [HD, HD], BF16, kind="ExternalInput")
    eye = nc.dram_tensor("eye", [HD, HD], BF16, kind="ExternalInput")
    ones2d = nc.dram_tensor("ones2d", [HD, HD], BF16, kind="ExternalInput")
    ones2df = nc.dram_tensor("ones2df", [HD, HD], F32, kind="ExternalInput")
    eyef = nc.dram_tensor("eyef", [HD, HD], F32, kind="ExternalInput")
    wo2 = nc.dram_tensor("wo2", [2, HD, D], BF16, kind="ExternalInput")
    y_own = nc.dram_tensor("y_own", [N, D], F32, kind="ExternalOutput")
    y_sh = nc.dram_tensor("y_sh", [NS, D], F32, kind="ExternalOutput")

    with tile.TileContext(nc) as tc:
        import contextlib

        stack = contextlib.ExitStack()
        with stack:
            persist = stack.enter_context(tc.tile_pool(name="persist", bufs=1))
            qT = persist.tile([128, NQ], BF16, tag="qT")
            kT = persist.tile([128, NK], BF16, tag="kT")
            vtok = persist.tile([128, 2, 25, HD], BF16, tag="vtok")
            aoT = persist.tile([128, NQ], BF16, tag="aoT")
            bias_sb = persist.tile([HD, 6], F32, tag="bias")
            nc.sync.dma_start(bias_sb[:], bias6[:])
            ones_sb = persist.tile([HD, HD], BF16, tag="ones")
            rot_sb = persist.tile([HD, HD], BF16, tag="rot")
            eye_sb = persist.tile([HD, HD], BF16, tag="eye")
            ones_f = persist.tile([HD, HD], F32, tag="onesf")
            eye_f = persist.tile([HD, HD], F32, tag="eyef")
            wo_sb = [persist.tile([HD, D], BF16, tag=f"wo{u}", name=f"wo_sb{u}")
                     for u in range(2)]
            tabs = {}
            for nm in ("cq", "sq", "ck", "sk"):
                tabs[nm] = persist.tile([128, N], BF16, tag=f"tab_{nm}",
                                        name=f"tab_{nm}")

            def load_aux():
                # issued after the first projection tiles so the Sync/DMA
                # queue prioritizes getting the PE started
                nc.sync.dma_start(ones_sb[:], ones2d[:])
                nc.sync.dma_start(rot_sb[:], rotm[:])
                nc.sync.dma_start(eye_sb[:], eye[:])
                nc.sync.dma_start(ones_f[:], ones2df[:])
                nc.sync.dma_start(eye_f[:], eyef[:])
                for u in range(2):
                    nc.sync.dma_start(wo_sb[u][:], wo2[u, :, :])
                for nm, dr in (("cq", cq), ("sq", sq), ("ck", ck), ("sk", sk)):
                    nc.sync.dma_start(tabs[nm][:], dr[:])
            rsk_sb = persist.tile([128, 50], F32, tag="rsk")
            inv_sb = persist.tile([128, NCHUNK], F32, tag="inv")
            rows_all = persist.tile([1, NCHUNK * 128], F32, tag="rows_all")
            rsq_all = persist.tile([128, NCHUNK], F32, tag="rsq_all")
            bias_q = persist.tile([128, 1], F32, tag="bias_q")
            nc.vector.memset(bias_q[:], HD * EPS)
            bias_k = persist.tile([128, 1], F32, tag="bias_k")
            nc.vector.memset(bias_k[:], EPS)

            row_ps = stack.enter_context(
                tc.tile_pool(name="row_ps", bufs=1, space="PSUM"))   # [1,512]
            smt_ps = stack.enter_context(
                tc.tile_pool(name="smt_ps", bufs=1, space="PSUM"))   # [128,8]
            rtmp = stack.enter_context(tc.tile_pool(name="rtmp", bufs=3))
            pools = {}

            def rms_rs(kind, src_row_psum, w, dst_f32, dst_col0, uid):
                """Transpose ssq row [1,w] onto partitions, rsqrt there."""
                ssq_row = rtmp.tile([1, RW], F32, tag="ssqr", name=f"sr{uid}")
                nc.scalar.activation(ssq_row[:, :w], src_row_psum, AF.Copy)
                smt = smt_ps.tile([128, 8], F32, tag="smt", name=f"smt{uid}")
                ncc = (w + 127) // 128
                for i in range(ncc):
                    lo = i * 128
                    ccw = min(128, w - lo)
                    nc.tensor.matmul(
                        smt[0:ccw, 2 * i:2 * i + 2],
                        ssq_row[0:1, lo:lo + ccw],
                        ones_f[0:1, 0:2],
                        start=True, stop=True,
                    )
                sq_t = rtmp.tile([128, 8], F32, tag="sqt", name=f"sq{uid}")
                if kind == "q":
                    nc.scalar.activation(
                        sq_t[:, :ncc], smt[:, 0:2 * ncc:2], AF.Sqrt,
                        bias=bias_q[:], scale=1.0,
                    )
                else:
                    nc.scalar.activation(
                        sq_t[:, :ncc], smt[:, 0:2 * ncc:2], AF.Sqrt,
                        bias=bias_k[:], scale=1.0 / HD,
                    )
                nc.vector.reciprocal(
                    dst_f32[:, dst_col0:dst_col0 + ncc], sq_t[:, :ncc])
                return ncc

            def rope_stats(kind, o, w, uid):
                """Sum-of-squares -> rs for one tile; q also flattens rs to
                a row in rows_all (for the later broadcast matmul)."""
                big = qT if kind == "q" else kT
                src = big[:, o:o + w]
                q2 = rtmp.tile([128, RW], BF16, tag="q2", name=f"q2{uid}")
                # SBUF-only op; Pool engine is idle before the chunk loop
                nc.gpsimd.tensor_mul(q2[:, :w], src, src)
                ssq = row_ps.tile([1, RW], F32, tag="row", name=f"ssq{uid}")
                nc.tensor.matmul(
                    ssq[:, :w], ones_sb[:, 0:1], q2[:, :w],
                    start=True, stop=True,
                )
                if kind == "k":
                    rms_rs("k", ssq[:, :w], w, rsk_sb, o // 128, uid)
                    return
                rms_rs("q", ssq[:, :w], w, rsq_all, o // 128, uid)

            def flatten_rs(o, w, uid):
                """rsq_all columns -> a row segment of rows_all."""
                g0 = o // 128
                ncc = (w + 127) // 128
                trp = pools["trp"].tile([128, 128], F32, tag="trp",
                                        name=f"trp{uid}")
                nc.tensor.transpose(
                    trp[0:ncc, :], rsq_all[:, g0:g0 + ncc], eye_f[:])
                rows8 = rtmp.tile([8, 128], F32, tag="rows8", name=f"r8{uid}")
                nc.vector.tensor_copy(rows8[0:ncc, :], trp[0:ncc, :])
                nc.sync.dma_start(
                    rows_all[0:1, o:o + ncc * 128], rows8[0:ncc, :])

            def rope_apply(kind, o, tok, w, uid, ps512):
                big = qT if kind == "q" else kT
                ctab = tabs["cq"] if kind == "q" else tabs["ck"]
                stab = tabs["sq"] if kind == "q" else tabs["sk"]
                src = big[:, o:o + w]
                rot = ps512.tile([128, RW], F32, tag="ps", name=f"rt{uid}")
                nc.tensor.matmul(
                    rot[:, :w], rot_sb[:], src, start=True, stop=True)
                if kind == "q":
                    bcp = ps512.tile([128, RW], F32, tag="ps", name=f"bc{uid}")
                    ncc = (w + 127) // 128
                    for i in range(ncc):
                        lo = i * 128
                        ccw = min(128, w - lo)
                        nc.tensor.matmul(
                            bcp[:, lo:lo + ccw], ones_f[0:1, :],
                            rows_all[0:1, o + lo:o + lo + ccw],
                            start=True, stop=True,
                        )
                m1 = rtmp.tile([128, RW], BF16, tag="m1", name=f"m1{uid}")
                if kind == "k":
                    nc.gpsimd.tensor_mul(m1[:, :w], src, ctab[:, tok:tok + w])
                else:
                    nc.vector.tensor_mul(m1[:, :w], src, ctab[:, tok:tok + w])
                m2 = rtmp.tile([128, RW], BF16, tag="m2", name=f"m2{uid}")
                nc.vector.tensor_mul(m2[:, :w], rot[:, :w], stab[:, tok:tok + w])
                if kind == "k":
                    nc.vector.tensor_add(src, m1[:, :w], m2[:, :w])
                else:
                    qr = rtmp.tile([128, RW], BF16, tag="qr", name=f"qr{uid}")
                    nc.vector.tensor_add(qr[:, :w], m1[:, :w], m2[:, :w])
                    nc.vector.tensor_mul(src, qr[:, :w], bcp[:, :w])

            # ---------------- projection phase ----------------
            with tc.tile_pool(name="vt", bufs=1) as vt_pool:
                vT = vt_pool.tile([128, NK], BF16, tag="vT")
                with tc.tile_pool(name="xt", bufs=1) as xt_pool, \
                     tc.tile_pool(name="wld", bufs=3) as w_pool, \
                     tc.tile_pool(name="pp", bufs=5, space="PSUM") as pp:
                    wt_next = None
                    for half in range(2):
                        h0 = half * 1600
                        if half == 0:
                            # first weights ahead of the big x transfers so
                            # the PE starts as soon as x chunk 0 lands
                            wt_next = w_pool.tile([128, NCH, HD], BF16,
                                                  tag="w", name="wt_first")
                            nc.sync.dma_start(wt_next[:], w6[0, :, :, :])
                        xts = []
                        for c in range(NCH):
                            xt = xt_pool.tile([128, 1600], BF16, tag=f"xt{c}")
                            nc.sync.dma_start(
                                xt[:], xT[c * 128:(c + 1) * 128, h0:h0 + 1600])
                            xts.append(xt)
                        if half == 1:
                            # aux tables land during half-1 compute, well
                            # before rope/out-proj need them, without delaying
                            # any projection weight loads
                            load_aux()
                        # blocks: 0 q_own, 1 q_sh, 2 k_own, 3 k_sh, 4 v_own, 5 v_sh
                        for b in range(6):
                            if b == 1 and half == 1:
                                continue
                            if b == 0:
                                dst, d0 = qT, h0
                            elif b == 1:
                                dst, d0 = qT, N + h0
                            elif b in (2, 3):
                                dst, d0 = kT, (b - 2) * N + h0
                            else:
                                dst, d0 = vT, (b - 4) * N + h0
                            if wt_next is not None:
                                wt_all, wt_next = wt_next, None
                            else:
                                wt_all = w_pool.tile([128, NCH, HD], BF16,
                                                     tag="w",
                                                     name=f"wt{half}_{b}")
                                nc.sync.dma_start(wt_all[:], w6[b, :, :, :])
                            # c-outer with 4 parallel psum tiles: each
                            # stationary loads once per block (12 LDW instead
                            # of 48), keeping the PE at stream rate
                            tiles4 = subtiles(1600, PW)
                            pss = [pp.tile([128, PW], F32, tag="pp",
                                           name=f"pp{half}_{b}_{oi}")
                                   for oi in range(len(tiles4))]
                            for c in range(NCH):
                                for oi, (o, w) in enumerate(tiles4):
                                    nc.tensor.matmul(
                                        pss[oi][:, :w], wt_all[:, c, :],
                                        xts[c][:, o:o + w],
                                        start=(c == 0), stop=(c == NCH - 1),
                                    )
                            for oi, (o, w) in enumerate(tiles4):
                                nc.vector.tensor_scalar_add(
                                    dst[:, d0 + o:d0 + o + w], pss[oi][:, :w],
                                    bias_sb[:, b:b + 1],
                                )

                k_tiles = []
                for seg in range(2):
                    for (ol, w) in subtiles(N, RW):
                        k_tiles.append((seg * N + ol, ol, w))
                chunks_pre = []
                for (ol, w) in subtiles(N, RW):
                    chunks_pre.append((0, ol, ol, w))     # unit, qcol, tok, w
                for (ol, w) in subtiles(NS, RW):
                    chunks_pre.append((1, N + ol, ol, w))

                with tc.tile_pool(name="ps512", bufs=4, space="PSUM") as ps512:
                    # V transposes (5 per psum bank) interleaved with rope(k)
                    with tc.tile_pool(name="vtp", bufs=2, space="PSUM") as vtp:
                        for i in range(len(k_tiles)):
                            if i < 10:
                                h, g = divmod(i, 5)
                                tpg = vtp.tile([128, 5, HD], BF16, tag="tp",
                                               name=f"tp{i}")
                                for k5 in range(5):
                                    jt = g * 5 + k5
                                    nc.tensor.transpose(
                                        tpg[:, k5, :],
                                        vT[:, h * N + jt * 128:
                                           h * N + (jt + 1) * 128],
                                        eye_sb[:],
                                    )
                                nc.scalar.activation(
                                    vtok[:, h, g * 5:(g + 1) * 5, :], tpg[:],
                                    AF.Copy)
                            (o, tok, w) = k_tiles[i]
                            rope_stats("k", o, w, f"k{i}")
                            rope_apply("k", o, tok, w, f"k{i}", ps512)
                            if i < len(chunks_pre):
                                rope_stats("q", chunks_pre[i][1],
                                           chunks_pre[i][3], f"q{i}")

                    chunks = chunks_pre

                    # flatten per-chunk rs columns into rows_all segments;
                    # the transpose psum bank frees before aops opens
                    with tc.tile_pool(name="trp_ps", bufs=1,
                                      space="PSUM") as trp_ps:
                        pools["trp"] = trp_ps
                        for ci, (unit, gco, tok, cw) in enumerate(chunks):
                            flatten_rs(gco, cw, f"q{ci}")

                    with tc.tile_pool(name="aops", bufs=2, space="PSUM") as aops, \
                         tc.tile_pool(name="expp", bufs=6) as expp, \
                         tc.tile_pool(name="accp", bufs=2) as accp, \
                         tc.tile_pool(name="yout", bufs=3) as yout:

                        state = {}

                        def attention(ci, unit, gco, cw):
                            head = unit
                            ao = aops.tile([128, RW], F32, tag="ao",
                                           name=f"ao{ci}")
                            # bf16 accumulators: halves the dominant SBUF
                            # traffic of the sum chains (loop is SBUF-BW
                            # bound); ~12 bf16 adds adds ~0.7% sum error
                            acc_a = accp.tile([128, RW], BF16, tag="acca",
                                              name=f"acca{ci}")
                            acc_b = accp.tile([128, RW], BF16, tag="accb",
                                              name=f"accb{ci}")
                            exs = {}
                            prev = None
                            for jt in range(25):
                                gjt = head * 25 + jt
                                sc = ps512.tile([128, RW], F32, tag="ps",
                                                name=f"sc{ci}_{jt}")
                                nc.tensor.matmul(
                                    sc[:, :cw],
                                    kT[:, gjt * 128:(gjt + 1) * 128],
                                    qT[:, gco:gco + cw],
                                    start=True, stop=True,
                                )
                                ex = expp.tile([128, RW], BF16, tag="ex",
                                               name=f"ex{ci}_{jt}")
                                nc.scalar.activation(
                                    ex[:, :cw], sc[:, :cw], AF.Exp,
                                    scale=rsk_sb[:, gjt:gjt + 1],
                                )
                                exs[jt] = ex
                                # softmax sums: two parallel f32 chains; Pool
                                # is slower per op so it gets the 3:2 share
                                # that equalizes finish times
                                if jt in (0, 1):
                                    pass  # consumed by the pair-starts below
                                elif jt == 2:
                                    nc.gpsimd.tensor_add(
                                        acc_a[:, :cw], exs[0][:, :cw],
                                        exs[2][:, :cw])
                                elif jt == 3:
                                    nc.vector.tensor_add(
                                        acc_b[:, :cw], exs[1][:, :cw],
                                        exs[3][:, :cw])
                                elif jt % 2 == 0:
                                    nc.gpsimd.tensor_add(
                                        acc_a[:, :cw], acc_a[:, :cw],
                                        ex[:, :cw])
                                else:
                                    nc.vector.tensor_add(
                                        acc_b[:, :cw], acc_b[:, :cw],
                                        ex[:, :cw])
                                if prev is not None:
                                    pex, pjt = prev
                                    nc.tensor.matmul(
                                        ao[:, :cw], vtok[:, head, pjt, :],
                                        pex[:, :cw],
                                        start=(pjt == 0), stop=False,
                                    )
                                prev = (ex, jt)
                            pex, pjt = prev
                            nc.tensor.matmul(
                                ao[:, :cw], vtok[:, head, pjt, :], pex[:, :cw],
                                start=False, stop=True,
                            )
                            state[ci] = (ao, acc_a, acc_b)

                        def tail(ci, unit, gco, tok, cw):
                            ao, acc_a, acc_b = state.pop(ci)
                            g0 = gco // 128
                            # both chains reduce into one psum row: no merge
                            # op, no merged-accumulator SBUF traffic
                            sm = row_ps.tile([1, RW], F32, tag="row",
                                             name=f"sm{ci}")
                            nc.tensor.matmul(
                                sm[:, :cw], ones_sb[:, 0:1], acc_a[:, :cw],
                                start=True, stop=False,
                            )
                            nc.tensor.matmul(
                                sm[:, :cw], ones_sb[:, 0:1], acc_b[:, :cw],
                                start=False, stop=True,
                            )
                            smrow = rtmp.tile([1, RW], F32, tag="ssqr",
                                              name=f"smr{ci}")
                            nc.vector.tensor_copy(smrow[:, :cw], sm[:, :cw])
                            smt = smt_ps.tile([128, 8], F32, tag="smt",
                                              name=f"smT{ci}")
                            ncc = (cw + 127) // 128
                            for i in range(ncc):
                                lo = i * 128
                                ccw = min(128, cw - lo)
                                nc.tensor.matmul(
                                    smt[0:ccw, 2 * i:2 * i + 2],
                                    smrow[0:1, lo:lo + ccw],
                                    ones_f[0:1, 0:2],
                                    start=True, stop=True,
                                )
                            nc.vector.reciprocal(
                                inv_sb[:, g0:g0 + ncc], smt[:, 0:2 * ncc:2])
                            nc.vector.tensor_copy(
                                aoT[:, gco:gco + cw], ao[:, :cw])

                        def outproj(ci, unit, gco, tok, cw):
                            ydst = y_own if unit == 0 else y_sh
                            for (it, iw) in subtiles(cw, 128):
                                git = gco + it
                                gidx = git // 128
                                yt = yout.tile([128, D], F32, tag="yt",
                                               name=f"yt{ci}_{it}")
                                for ct3 in range(3):
                                    op = ps512.tile([128, RW], F32, tag="ps",
                                                    name=f"op{ci}_{it}_{ct3}")
                                    nc.tensor.matmul(
                                        op[0:iw, :], aoT[:, git:git + iw],
                                        wo_sb[unit][:, ct3 * 512:(ct3 + 1) * 512],
                                        start=True, stop=True,
                                    )
                                    if (ct3 + it // 128) % 2 == 0:
                                        nc.vector.tensor_scalar_mul(
                                            yt[0:iw, ct3 * 512:(ct3 + 1) * 512],
                                            op[0:iw, :],
                                            inv_sb[0:iw, gidx:gidx + 1],
                                        )
                                    else:
                                        # Copy lives in every ACT table: no
                                        # act-table swap against Exp
                                        nc.scalar.activation(
                                            yt[0:iw, ct3 * 512:(ct3 + 1) * 512],
                                            op[0:iw, :], AF.Copy,
                                            scale=inv_sb[0:iw, gidx:gidx + 1],
                                        )
                                nc.sync.dma_start(
                                    ydst[tok + it:tok + it + iw, :], yt[0:iw, :])

                        # tail(ci-1) lands at the head of iteration ci so its
                        # psum drains finish under rope/attention PE work;
                        # outproj(ci-1) runs after attention(ci) when inv and
                        # aoT are long ready -- the PE never stalls on them
                        for ci, (unit, gco, tok, cw) in enumerate(chunks):
                            rope_apply("q", gco, tok, cw, f"q{ci}", ps512)
                            if ci > 0:
                                tail(ci - 1, *chunks[ci - 1])
                            attention(ci, unit, gco, cw)
                            if ci > 0:
                                outproj(ci - 1, *chunks[ci - 1])
                        tail(len(chunks) - 1, *chunks[-1])
                        outproj(len(chunks) - 1, *chunks[-1])

    nc.compile()
    return nc


def _get_nc():
    if "nc" not in _CACHE:
        _CACHE["nc"] = _build()
    return _CACHE["nc"]


def _host_prep(inputs):
    import ml_dtypes

    bf16 = ml_dtypes.bfloat16
    x = np.asarray(inputs["x"], np.float32)[0]          # [N, D]
    Wq = np.asarray(inputs["Wq"], np.float32)
    Wk = np.asarray(inputs["Wk"], np.float32)
    Wv = np.asarray(inputs["Wv"], np.float32)
    Wo = np.asarray(inputs["Wo"], np.float32)
    bq = np.asarray(inputs["bq"], np.float32)
    bk = np.asarray(inputs["bk"], np.float32)
    bv = np.asarray(inputs["bv"], np.float32)
    qs = np.asarray(inputs["q_scale"], np.float32)
    ks = np.asarray(inputs["k_scale"], np.float32)
    ft = np.asarray(inputs["freqs_t"], np.float32)
    fh = np.asarray(inputs["freqs_h"], np.float32)
    fw = np.asarray(inputs["freqs_w"], np.float32)

    cos = np.zeros((N, HD // 2), np.float32)
    sin = np.zeros((N, HD // 2), np.float32)
    idx = np.arange(N)
    f_idx, h_idx, w_idx = idx // (Hg * Wg), (idx // Wg) % Hg, idx % Wg
    cos[:, 0:22], sin[:, 0:22] = ft[f_idx, :, 0], ft[f_idx, :, 1]
    cos[:, 22:43], sin[:, 22:43] = fh[h_idx, :, 0], fh[h_idx, :, 1]
    cos[:, 43:64], sin[:, 43:64] = fw[w_idx, :, 0], fw[w_idx, :, 1]
    C = np.repeat(cos, 2, axis=1).T.copy()               # [128, N]
    S = np.repeat(sin, 2, axis=1).T.copy()
    qs_sw = qs.reshape(64, 2)[:, ::-1].reshape(128)
    ks_sw = ks.reshape(64, 2)[:, ::-1].reshape(128)
    Cq, Sq = C * qs[:, None], S * qs_sw[:, None]
    Ck, Sk = C * ks[:, None], S * ks_sw[:, None]

    rotm = np.zeros((128, 128), np.float32)
    pr = np.arange(64)
    rotm[2 * pr + 1, 2 * pr] = -1.0
    rotm[2 * pr, 2 * pr + 1] = 1.0
    eye = np.eye(128, dtype=np.float32)
    ones2d = np.ones((128, 128), np.float32)

    xT = np.ascontiguousarray(x.T)                       # [D, N]
    perm_swap = np.concatenate([np.arange(1600, N), np.arange(0, 1600)])

    in_maps = []
    for core in range(8):
        pair, parity = core // 2, core % 2
        own, sh = 3 * pair + parity, 3 * pair + 2
        if parity == 0:
            xTc, Cqc, Sqc, Ckc, Skc = xT, Cq, Sq, Ck, Sk
        else:
            xTc = np.ascontiguousarray(xT[:, perm_swap])
            Cqc = np.ascontiguousarray(Cq[:, perm_swap])
            Sqc = np.ascontiguousarray(Sq[:, perm_swap])
            Ckc = np.ascontiguousarray(Ck[:, perm_swap])
            Skc = np.ascontiguousarray(Sk[:, perm_swap])
        w6 = np.stack([
            Wq[:, own * HD:(own + 1) * HD], Wq[:, sh * HD:(sh + 1) * HD],
            Wk[:, own * HD:(own + 1) * HD], Wk[:, sh * HD:(sh + 1) * HD],
            Wv[:, own * HD:(own + 1) * HD], Wv[:, sh * HD:(sh + 1) * HD],
        ])
        # [6, D, HD] -> [6, 128, NCH, HD] so each block loads in ONE dma
        w6 = w6.reshape(6, NCH, 128, HD).transpose(0, 2, 1, 3)
        bias6 = np.stack([
            bq[own * HD:(own + 1) * HD], bq[sh * HD:(sh + 1) * HD],
            bk[own * HD:(own + 1) * HD], bk[sh * HD:(sh + 1) * HD],
            bv[own * HD:(own + 1) * HD], bv[sh * HD:(sh + 1) * HD],
        ], axis=1)
        wo2 = np.stack([
            Wo[own * HD:(own + 1) * HD, :], Wo[sh * HD:(sh + 1) * HD, :],
        ])
        in_maps.append({
            "xT": xTc.astype(bf16), "w6": np.ascontiguousarray(w6).astype(bf16),
            "bias6": np.ascontiguousarray(bias6),
            "cq": Cqc.astype(bf16), "sq": Sqc.astype(bf16),
            "ck": Ckc.astype(bf16), "sk": Skc.astype(bf16),
            "rotm": rotm.astype(bf16), "eye": eye.astype(bf16),
            "ones2d": ones2d.astype(bf16), "ones2df": ones2d, "eyef": eye,
            "wo2": np.ascontiguousarray(wo2).astype(bf16),
        })
    return in_maps, perm_swap


def _gather(results, perm_swap, bo):
    inv_swap = perm_swap  # swapping halves is its own inverse
    y = np.zeros((N, D), np.float32)
    for core in range(8):
        parity = core % 2
        yo = np.asarray(results[core]["y_own"], np.float32)
        ysh = np.asarray(results[core]["y_sh"], np.float32)
        if parity == 0:
            y += yo
            y[0:1600] += ysh
        else:
            y += yo[inv_swap]
            y[1600:3200] += ysh
    y += bo[None, :]
    return y[None]


def run_internal(inputs, trace=False, **kw):
    from concourse.bass_utils import run_bass_kernel_spmd

    nc = _get_nc()
    in_maps, perm_swap = _host_prep(inputs)
    res = run_bass_kernel_spmd(
        nc, in_maps, core_ids=list(range(8)), trace=trace, **kw
    )
    bo = np.asarray(inputs["bo"], np.float32)
    y = _gather(res.results, perm_swap, bo)
    return y, res


def kernel(**inputs):
    y, _ = run_internal(inputs, trace=False)
    return y
